# revision 1
# baseline (speedup 1.0000x reference)
"""Trainium2 Bass kernel for nn_Net_39041252721137 (supermask MLP with global
top-50% |score| masking).

Data-parallel on batch across 8 cores; replicated scores/weights. Exact
global top-k thresholds computed ON DEVICE per core:
  P1   coarse stratified count on a resident subset (one k-tile)
  P1.5 fine stratified count streaming the full tensor
  P2   exact count below bracket + band compaction (iterated DVE
       max8/match_replace top-40 per row-chunk; all-ISA, no gpsimd ucode)
  P3   exact float-space bisection over the compacted band
Then masked bf16 matmuls: h = relu(x @ (w1*m1).T), logits = h @ (w2*m2).T,
log_softmax, fused in one pass over neuron blocks.
"""
import sys

import numpy as np
import ml_dtypes

sys.path.insert(0, "/root/.axon_site")

import concourse.bass as bass
import concourse.bacc as bacc
import concourse.mybir as mybir
import concourse.tile as tile
from concourse.bass_isa import ReduceOp
from concourse.bass_utils import run_bass_kernel_spmd
from concourse.masks import make_identity

F32 = mybir.dt.float32
BF16 = mybir.dt.bfloat16
U32 = mybir.dt.uint32
AF = mybir.ActivationFunctionType
ALU = mybir.AluOpType
AX = mybir.AxisListType

N_CORES = 8
B, D_IN, N2, N_OUT = 16384, 784, 8192, 10
BS = B // N_CORES            # 2048 batch rows per core
KT, KP = 7, 112              # d_in tiled as 7 x 112 partitions
N1 = N2 * D_IN               # 6422528
J1 = N1 // 2
NS2 = N_OUT * N2             # 81920
J2 = NS2 // 2
M0 = 210_000                 # coarse bracket margin (ranks)
M2 = 25_000                  # fine bracket margin (ranks)
R1 = 21                      # s1 bisection rounds (offline: isolates by 16)
R2 = 24                      # s2 bisection rounds (offline: isolates by 18)
NB = N2 // 128               # 64 neuron blocks
BBS = 512
NBB = BS // BBS              # 4
CH = 2048                    # s1 streaming chunk width
NCH = N2 // CH               # 4 chunks per k-tile, 28 total
GW = 128                     # stage-1 gather output width per call
NEG_BITS = float(0xFF800000)  # -inf bitpattern; exactly representable in f32

_cache = {}


def _bisect(nc, pool, vals_ap, P, n_free, lo, hi, j_ap, rounds, ones_b):
    """Float-space bisection for the rank-j value (ascending, 0-indexed).

    vals_ap: [P, F] f32 data (sentinels must be negative, below initial lo>=0
    or excluded because lo starts > them). lo/hi: [P,1] f32 tiles, all
    partitions equal, invariant c(<lo) <= j < c(<hi) where c counts vals
    plus the caller-folded base (j_ap = j - base). The midpoint updates
    lo/hi exactly (Sterbenz). After `rounds` the interval [lo, hi) contains
    exactly one data value (verified offline); extract with _extract."""
    for _ in range(rounds):
        m = pool.tile([P, 1], F32, tag="bis_m")
        nc.vector.tensor_tensor(m[:], lo[:], hi[:], op=ALU.add)
        nc.vector.tensor_scalar(m[:], m[:], 0.5, scalar2=None, op0=ALU.mult)
        scr = pool.tile([P, n_free], F32, tag="bis_scr")
        cnt = pool.tile([P, 1], F32, tag="bis_cnt")
        nc.vector.scalar_tensor_tensor(
            scr[:], vals_ap, m[:, :1], ones_b, op0=ALU.is_lt, op1=ALU.mult,
            accum_out=cnt[:])
        tot = pool.tile([P, 1], F32, tag="bis_tot")
        nc.gpsimd.partition_all_reduce(tot[:], cnt[:], channels=P,
                                       reduce_op=ReduceOp.add)
        pred = pool.tile([P, 1], F32, tag="bis_pred")
        nc.vector.tensor_tensor(pred[:], tot[:], j_ap, op=ALU.is_le)
        npred = pool.tile([P, 1], F32, tag="bis_npred")
        nc.vector.tensor_scalar(npred[:], pred[:], -1.0, scalar2=1.0,
                                op0=ALU.mult, op1=ALU.add)
        # lo += (m - lo)*pred ; hi -= (hi - m)*(1 - pred)   (all exact)
        t1 = pool.tile([P, 1], F32, tag="bis_t1")
        nc.vector.tensor_tensor(t1[:], m[:], lo[:], op=ALU.subtract)
        nc.vector.tensor_tensor(t1[:], t1[:], pred[:], op=ALU.mult)
        nc.vector.tensor_tensor(lo[:], lo[:], t1[:], op=ALU.add)
        t2 = pool.tile([P, 1], F32, tag="bis_t2")
        nc.vector.tensor_tensor(t2[:], hi[:], m[:], op=ALU.subtract)
        nc.vector.tensor_tensor(t2[:], t2[:], npred[:], op=ALU.mult)
        nc.vector.tensor_tensor(hi[:], hi[:], t2[:], op=ALU.subtract)


def _extract(nc, pool, st, vals_ap, P, n_free, lo, hi, name):
    """v = the unique data value in [lo, hi): max over vals of v*pred."""
    p1 = pool.tile([P, n_free], F32, tag="bis_scr")
    nc.vector.tensor_scalar(p1[:], vals_ap, hi[:, :1], scalar2=None,
                            op0=ALU.is_lt)
    pm = pool.tile([P, n_free], F32, tag="bis_scr2")
    nc.vector.scalar_tensor_tensor(
        pm[:], vals_ap, lo[:, :1], p1[:], op0=ALU.is_ge, op1=ALU.mult)
    nc.vector.tensor_tensor(pm[:], pm[:], vals_ap, op=ALU.mult)
    vrow = pool.tile([P, 1], F32, tag="bis_vrow")
    nc.vector.tensor_reduce(vrow[:], pm[:], axis=AX.X, op=ALU.max)
    v = st.tile([P, 1], F32, name=name, tag=name)
    nc.gpsimd.partition_all_reduce(v[:], vrow[:], channels=P,
                                   reduce_op=ReduceOp.max)
    return v


def build_program():
    nc = bacc.Bacc("TRN2", target_bir_lowering=False, debug=False,
                   num_devices=N_CORES)

    xT = nc.declare_dram_parameter("xT", [KT, KP, BS], BF16, isOutput=False)
    w1T = nc.declare_dram_parameter("w1T", [KT, KP, N2], BF16, isOutput=False)
    s1T = nc.declare_dram_parameter("s1T", [KT, KP, N2], F32, isOutput=False)
    w2r = nc.declare_dram_parameter("w2r", [128, NB * N_OUT], BF16, isOutput=False)
    s2r = nc.declare_dram_parameter("s2r", [128, NB * N_OUT], F32, isOutput=False)
    out = nc.declare_dram_parameter("out", [BS, N_OUT], F32, isOutput=True)

    with tile.TileContext(nc) as tc:
        with (
            tc.tile_pool(name="state", bufs=1) as st,
            tc.tile_pool(name="bis", bufs=2) as bis,
            tc.tile_pool(name="stream", bufs=2) as strm,
            tc.tile_pool(name="nfp", bufs=2) as nfp,
            tc.tile_pool(name="band", bufs=1) as bandp,
            tc.tile_pool(name="mm", bufs=3) as mmp,
            tc.tile_pool(name="hbuf", bufs=8) as hbp,
            tc.tile_pool(name="psum_h", bufs=4, space="PSUM") as psh,
            tc.tile_pool(name="psum_l", bufs=1, space="PSUM") as psl,
            tc.tile_pool(name="epi", bufs=2) as epi,
        ):
            # ---- shared constants ----
            onef = st.tile([128, 1], F32)
            nc.vector.memset(onef[:], 1.0)
            zb = st.tile([128, 1], F32)
            nc.vector.memset(zb[:], 0.0)
            zbf16 = st.tile([128, 1], BF16)
            nc.vector.memset(zbf16[:], 0.0)
            negf = st.tile([128, 1], F32)
            nc.vector.memset(negf[:], -1.0)
            ident = st.tile([128, 128], F32)
            make_identity(nc, ident[:])

            # ================= s2 threshold =================
            s2sb = st.tile([128, NB * N_OUT], F32)
            nc.sync.dma_start(s2sb[:], s2r[:])
            a2 = st.tile([128, NB * N_OUT], F32)
            nc.vector.tensor_scalar(a2[:].bitcast(U32), s2sb[:].bitcast(U32),
                                    0x7FFFFFFF, scalar2=None, op0=ALU.bitwise_and)
            lo2 = st.tile([128, 1], F32)
            nc.vector.memset(lo2[:], 0.0)
            rm2 = st.tile([128, 1], F32)
            nc.vector.tensor_reduce(rm2[:], a2[:], axis=AX.X, op=ALU.max)
            hi2 = st.tile([128, 1], F32)
            nc.gpsimd.partition_all_reduce(hi2[:], rm2[:], channels=128,
                                           reduce_op=ReduceOp.max)
            j2t = st.tile([128, 1], F32)
            nc.vector.memset(j2t[:], float(J2))
            _bisect(nc, bis, a2[:], 128, NB * N_OUT, lo2, hi2, j2t[:], R2,
                    onef[:].to_broadcast([128, NB * N_OUT]))
            v2 = _extract(nc, bis, st, a2[:], 128, NB * N_OUT, lo2, hi2, "v2")
            # w2 masked: keep where |s2| >= v2
            pr2 = st.tile([128, NB * N_OUT], U32)
            nc.vector.tensor_scalar(pr2[:], a2[:], v2[:, :1], scalar2=None,
                                    op0=ALU.is_lt)
            w2raw = st.tile([128, NB * N_OUT], BF16)
            nc.sync.dma_start(w2raw[:], w2r[:])
            w2m = st.tile([128, NB * N_OUT], BF16)
            nc.vector.select(w2m[:], pr2[:],
                             zbf16[:].to_broadcast([128, NB * N_OUT]), w2raw[:])

            # ================= s1 threshold =================
            # ---- P0: amax over subset = k-tile 0 (streamed) ----
            rmax = st.tile([KP, 1], F32)
            nc.vector.memset(rmax[:], 0.0)
            for c in range(NCH):
                t = strm.tile([KP, CH], F32, tag="s1t")
                nc.sync.dma_start(t[:], s1T[0][:, c * CH:(c + 1) * CH])
                cm = strm.tile([KP, 1], F32, tag="s1cm")
                nc.vector.tensor_reduce(cm[:], t[:], axis=AX.X, op=ALU.max,
                                        apply_absolute_value=True)
                nc.vector.tensor_tensor(rmax[:], rmax[:], cm[:], op=ALU.max)
            gmax = st.tile([KP, 1], F32)
            nc.gpsimd.partition_all_reduce(gmax[:], rmax[:], channels=KP,
                                           reduce_op=ReduceOp.max)
            iot = st.tile([KP, 1], F32)
            nc.gpsimd.iota(iot[:], pattern=[[0, 1]], base=1, channel_multiplier=1,
                           allow_small_or_imprecise_dtypes=True)
            gsc = st.tile([KP, 1], F32)
            nc.vector.tensor_scalar(gsc[:], gmax[:], 1.0 / KP, scalar2=None,
                                    op0=ALU.mult)
            grid1 = st.tile([KP, 1], F32)
            nc.vector.tensor_tensor(grid1[:], iot[:], gsc[:], op=ALU.mult)
            # ---- P1: coarse stratified count over the streamed subset ----
            acc1 = st.tile([KP, 1], F32)
            nc.vector.memset(acc1[:], 0.0)
            ones_ch = onef[:KP].to_broadcast([KP, CH])
            for c in range(NCH):
                t = strm.tile([KP, CH], F32, tag="s1t")
                nc.sync.dma_start(t[:], s1T[0][:, c * CH:(c + 1) * CH])
                a = strm.tile([KP, CH], F32, tag="s1a")
                nc.scalar.activation(a[:], t[:], AF.Abs, bias=0.0, scale=1.0)
                ck = strm.tile([KP, 1], F32, tag="s1ck")
                nc.vector.scalar_tensor_tensor(
                    t[:], a[:], grid1[:, :1], ones_ch, op0=ALU.is_lt,
                    op1=ALU.mult, accum_out=ck[:])
                nc.vector.tensor_tensor(acc1[:], acc1[:], ck[:], op=ALU.add)
            # chat1 = c_p * (N1/8192) = acc1 * 784
            chat1 = st.tile([KP, 1], F32)
            nc.vector.tensor_scalar(chat1[:], acc1[:], 784.0, scalar2=None,
                                    op0=ALU.mult)
            selL = st.tile([KP, 1], F32)
            nc.vector.tensor_scalar(selL[:], chat1[:], float(J1 - M0),
                                    scalar2=None, op0=ALU.is_lt)
            candL = st.tile([KP, 1], F32)
            nc.vector.tensor_tensor(candL[:], grid1[:], selL[:], op=ALU.mult)
            L0 = st.tile([KP, 1], F32)
            nc.gpsimd.partition_all_reduce(L0[:], candL[:], channels=KP,
                                           reduce_op=ReduceOp.max)
            selU = st.tile([KP, 1], U32)
            nc.vector.tensor_scalar(selU[:], chat1[:], float(J1 + M0),
                                    scalar2=None, op0=ALU.is_gt)
            candU = st.tile([KP, 1], F32)
            nc.vector.select(candU[:], selU[:], grid1[:], gmax[:])
            nc.vector.tensor_scalar(candU[:], candU[:], -1.0, scalar2=None,
                                    op0=ALU.mult)
            U0 = st.tile([KP, 1], F32)
            nc.gpsimd.partition_all_reduce(U0[:], candU[:], channels=KP,
                                           reduce_op=ReduceOp.max)
            nc.vector.tensor_scalar(U0[:], U0[:], -1.0, scalar2=None,
                                    op0=ALU.mult)

            # ---- P1.5: fine stratified, full stream ----
            grid2 = st.tile([KP, 1], F32)
            nc.vector.tensor_tensor(grid2[:], U0[:], L0[:], op=ALU.subtract)
            nc.vector.tensor_scalar(grid2[:], grid2[:], 1.0 / KP, scalar2=None,
                                    op0=ALU.mult)
            nc.vector.tensor_tensor(grid2[:], iot[:], grid2[:], op=ALU.mult)
            nc.vector.tensor_tensor(grid2[:], grid2[:], L0[:], op=ALU.add)
            accd = st.tile([KP, 1], F32)
            nc.vector.memset(accd[:], 0.0)
            for kt in range(KT):
                for c in range(NCH):
                    t = strm.tile([KP, CH], F32, tag="s1t")
                    nc.sync.dma_start(t[:], s1T[kt][:, c * CH:(c + 1) * CH])
                    a = strm.tile([KP, CH], F32, tag="s1a")
                    nc.scalar.activation(a[:], t[:], AF.Abs, bias=0.0,
                                         scale=1.0)
                    ck = strm.tile([KP, 1], F32, tag="s1ck")
                    nc.vector.scalar_tensor_tensor(
                        t[:], a[:], grid2[:, :1], ones_ch, op0=ALU.is_lt,
                        op1=ALU.mult, accum_out=ck[:])
                    nc.vector.tensor_tensor(accd[:], accd[:], ck[:], op=ALU.add)
            # chat2 = c_p * 112
            chat2 = st.tile([KP, 1], F32)
            nc.vector.tensor_scalar(chat2[:], accd[:], 112.0, scalar2=None,
                                    op0=ALU.mult)
            selL2 = st.tile([KP, 1], U32)
            nc.vector.tensor_scalar(selL2[:], chat2[:], float(J1 - M2),
                                    scalar2=None, op0=ALU.is_lt)
            candL2 = st.tile([KP, 1], F32)
            nc.vector.select(candL2[:], selL2[:], grid2[:], L0[:])
            L1 = st.tile([KP, 1], F32)
            nc.gpsimd.partition_all_reduce(L1[:], candL2[:], channels=KP,
                                           reduce_op=ReduceOp.max)
            selU2 = st.tile([KP, 1], U32)
            nc.vector.tensor_scalar(selU2[:], chat2[:], float(J1 + M2),
                                    scalar2=None, op0=ALU.is_gt)
            candU2 = st.tile([KP, 1], F32)
            nc.vector.select(candU2[:], selU2[:], grid2[:], U0[:])
            nc.vector.tensor_scalar(candU2[:], candU2[:], -1.0, scalar2=None,
                                    op0=ALU.mult)
            U1 = st.tile([KP, 1], F32)
            nc.gpsimd.partition_all_reduce(U1[:], candU2[:], channels=KP,
                                           reduce_op=ReduceOp.max)
            nc.vector.tensor_scalar(U1[:], U1[:], -1.0, scalar2=None,
                                    op0=ALU.mult)

            # ---- P2: exact count below L1 + band extraction ----
            # Mark out-of-band |s1| to -1.0, then compact via MX8 iterated
            # max8 + match_replace per chunk (pure DVE ISA — sparse_gather
            # ucode crashes on this hardware). Offline: max band elements
            # per (row, chunk) is ~30 of lambda=14.3, MX8*8 slots suffice.
            MX8 = 5
            accb = st.tile([KP, 1], F32)
            nc.vector.memset(accb[:], 0.0)
            B2 = bandp.tile([KP, KT * NCH * MX8 * 8], F32)
            for kt in range(KT):
                for c in range(NCH):
                    t = strm.tile([KP, CH], F32, tag="s1t")
                    nc.sync.dma_start(t[:], s1T[kt][:, c * CH:(c + 1) * CH])
                    a = strm.tile([KP, CH], F32, tag="s1a")
                    nc.scalar.activation(a[:], t[:], AF.Abs, bias=0.0,
                                         scale=1.0)
                    # olo (into t's buffer) = (a < L1), accum -> chunk count
                    olo = t[:].bitcast(U32)
                    ck = strm.tile([KP, 1], F32, tag="s1ck")
                    nc.vector.scalar_tensor_tensor(
                        olo, a[:], L1[:, :1], ones_ch, op0=ALU.is_lt,
                        op1=ALU.mult, accum_out=ck[:])
                    nc.vector.tensor_tensor(accb[:], accb[:], ck[:], op=ALU.add)
                    z = strm.tile([KP, CH], F32, tag="s1z")
                    zu = z[:].bitcast(U32)
                    # oob = (a >= U1) + olo  in {0,1}
                    nc.vector.scalar_tensor_tensor(
                        zu, a[:], U1[:, :1], olo, op0=ALU.is_ge, op1=ALU.add)
                    # out-of-band -> -1.0 (in place on the abs tile)
                    nc.vector.copy_predicated(
                        a[:], zu, negf[:KP].to_broadcast([KP, CH]))
                    # top-40 per row -> B2 slice; extracted values zapped to -2
                    base = (kt * NCH + c) * MX8 * 8
                    src = a
                    for i in range(MX8):
                        mx = B2[:, base + i * 8: base + (i + 1) * 8]
                        nc.vector.max(out=mx, in_=src[:])
                        if i < MX8 - 1:
                            nxt = strm.tile([KP, CH], F32,
                                            tag=("s1z" if i % 2 == 0 else "s1a"),
                                            name=f"mr{kt}_{c}_{i}")
                            nc.vector.match_replace(
                                out=nxt[:], in_to_replace=mx,
                                in_values=src[:], imm_value=-2.0)
                            src = nxt
            cbase = st.tile([KP, 1], F32)
            nc.gpsimd.partition_all_reduce(cbase[:], accb[:], channels=KP,
                                           reduce_op=ReduceOp.add)
            NB2 = KT * NCH * MX8 * 8
            # ---- P3: bisection on the compacted band ----
            # padding (-1/-2 maxima) counts as "< m" in float space; fold its
            # count into the rank target: j' = J1 - cbase + #padding.
            scrp = bis.tile([KP, NB2], F32, tag="bis_scr")
            cpk = st.tile([KP, 1], F32)
            nc.vector.scalar_tensor_tensor(
                scrp[:], B2[:], L1[:, :1], onef[:KP].to_broadcast([KP, NB2]),
                op0=ALU.is_lt, op1=ALU.mult, accum_out=cpk[:])
            cpad = st.tile([KP, 1], F32)
            nc.gpsimd.partition_all_reduce(cpad[:], cpk[:], channels=KP,
                                           reduce_op=ReduceOp.add)
            j1t = st.tile([KP, 1], F32)
            nc.vector.tensor_scalar(j1t[:], cbase[:], -1.0, scalar2=float(J1),
                                    op0=ALU.mult, op1=ALU.add)
            nc.vector.tensor_tensor(j1t[:], j1t[:], cpad[:], op=ALU.add)
            lo1 = st.tile([KP, 1], F32)
            hi1 = st.tile([KP, 1], F32)
            nc.vector.tensor_copy(lo1[:], L1[:])
            nc.vector.tensor_copy(hi1[:], U1[:])
            _bisect(nc, bis, B2[:], KP, NB2, lo1, hi1, j1t[:], R1,
                    onef[:KP].to_broadcast([KP, NB2]))
            v1 = _extract(nc, bis, st, B2[:], KP, NB2, lo1, hi1, "v1")

            # ================= matmul pipeline =================
            xsb = st.tile([KP, KT * BS], BF16)
            for kt in range(KT):
                nc.sync.dma_start(xsb[:, kt * BS:(kt + 1) * BS], xT[kt])

            lgps = [psl.tile([N_OUT, BBS], F32, tag=f"lg{bb}", name=f"lg{bb}")
                    for bb in range(NBB)]
            for nb in range(NB):
                w1b = mmp.tile([KP, KT * 128], BF16, tag="w1b")
                s1b = mmp.tile([KP, KT * 128], F32, tag="s1b")
                for kt in range(KT):
                    nc.sync.dma_start(
                        w1b[:, kt * 128:(kt + 1) * 128],
                        w1T[kt][:, nb * 128:(nb + 1) * 128])
                    nc.sync.dma_start(
                        s1b[:, kt * 128:(kt + 1) * 128],
                        s1T[kt][:, nb * 128:(nb + 1) * 128])
                prb = mmp.tile([KP, KT * 128], F32, tag="prb")
                nc.vector.tensor_scalar(prb[:].bitcast(U32), s1b[:].bitcast(U32),
                                        0x7FFFFFFF, scalar2=None,
                                        op0=ALU.bitwise_and)
                pru = mmp.tile([KP, KT * 128], U32, tag="pru")
                nc.vector.tensor_scalar(pru[:], prb[:], v1[:, :1], scalar2=None,
                                        op0=ALU.is_lt)
                w1m = mmp.tile([KP, KT * 128], BF16, tag="w1m")
                nc.vector.select(w1m[:], pru[:],
                                 zbf16[:KP].to_broadcast([KP, KT * 128]), w1b[:])
                hts = []
                for bb in range(NBB):
                    ph = psh.tile([128, BBS], F32, tag="ph")
                    for kt in range(KT):
                        nc.tensor.matmul(
                            ph[:], w1m[:, kt * 128:(kt + 1) * 128],
                            xsb[:, kt * BS + bb * BBS: kt * BS + (bb + 1) * BBS],
                            start=(kt == 0), stop=(kt == KT - 1))
                    ht = hbp.tile([128, BBS], BF16, tag="ht")
                    nc.scalar.activation(ht[:], ph[:], AF.Relu, bias=0.0,
                                         scale=1.0)
                    hts.append(ht)
                w2s = w2m[:, nb * N_OUT:(nb + 1) * N_OUT]
                for bb in range(NBB):
                    nc.tensor.matmul(lgps[bb][:], w2s, hts[bb][:],
                                     start=(nb == 0), stop=(nb == NB - 1),
                                     skip_group_check=True)

            # ================= epilogue: log_softmax =================
            for bb in range(NBB):
                lg = epi.tile([N_OUT, BBS], F32, tag="lg")
                nc.vector.tensor_copy(lg[:], lgps[bb][:])
                for c in range(BBS // 128):
                    pt = psh.tile([128, N_OUT], F32, tag="ph")
                    nc.tensor.transpose(pt[:, :N_OUT],
                                        lg[:, c * 128:(c + 1) * 128],
                                        ident[:N_OUT, :N_OUT])
                    lgt = epi.tile([128, N_OUT], F32, tag="lgt")
                    nc.vector.tensor_copy(lgt[:], pt[:])
                    mx = epi.tile([128, 1], F32, tag="mx")
                    nc.vector.tensor_reduce(mx[:], lgt[:], axis=AX.X, op=ALU.max)
                    nmx = epi.tile([128, 1], F32, tag="nmx")
                    nc.vector.tensor_scalar(nmx[:], mx[:], -1.0, scalar2=None,
                                            op0=ALU.mult)
                    ex = epi.tile([128, N_OUT], F32, tag="ex")
                    se = epi.tile([128, 1], F32, tag="se")
                    nc.scalar.activation(ex[:], lgt[:], AF.Exp, bias=nmx[:],
                                         scale=1.0, accum_out=se[:])
                    ls = epi.tile([128, 1], F32, tag="ls")
                    nc.scalar.activation(ls[:], se[:], AF.Ln, bias=zb[:, :1],
                                         scale=1.0)
                    tot = epi.tile([128, 1], F32, tag="lstot")
                    nc.vector.tensor_tensor(tot[:], mx[:], ls[:], op=ALU.add)
                    o = epi.tile([128, N_OUT], F32, tag="o")
                    nc.vector.tensor_scalar(o[:], lgt[:], tot[:, :1],
                                            scalar2=None, op0=ALU.subtract)
                    nc.sync.dma_start(
                        out[bb * BBS + c * 128: bb * BBS + (c + 1) * 128, :],
                        o[:])
    nc.compile()
    return nc


def _prep_inputs(x, w1, s1, w2, s2):
    bf = ml_dtypes.bfloat16
    w1T = np.ascontiguousarray(w1.T).reshape(KT, KP, N2).astype(bf)
    s1T = np.ascontiguousarray(s1.T).reshape(KT, KP, N2).astype(np.float32)
    w2r = np.ascontiguousarray(
        w2.T.reshape(NB, 128, N_OUT).transpose(1, 0, 2).reshape(128, NB * N_OUT)
    ).astype(bf)
    s2r = np.ascontiguousarray(
        s2.T.reshape(NB, 128, N_OUT).transpose(1, 0, 2).reshape(128, NB * N_OUT)
    ).astype(np.float32)
    in_maps = []
    for cid in range(N_CORES):
        xc = np.ascontiguousarray(
            x[cid * BS:(cid + 1) * BS].T).reshape(KT, KP, BS).astype(bf)
        in_maps.append({"xT": xc, "w1T": w1T, "s1T": s1T,
                        "w2r": w2r, "s2r": s2r})
    return in_maps


def kernel(x, w1, s1, w2, s2):
    x = np.asarray(x); w1 = np.asarray(w1); s1 = np.asarray(s1)
    w2 = np.asarray(w2); s2 = np.asarray(s2)
    if "nc" not in _cache:
        _cache["nc"] = build_program()
    nc = _cache["nc"]
    in_maps = _prep_inputs(x, w1, s1, w2, s2)
    res = run_bass_kernel_spmd(nc, in_maps, list(range(N_CORES)))
    return np.concatenate([res.results[c]["out"] for c in range(N_CORES)],
                          axis=0)


if __name__ == "__main__":
    sys.path.insert(0, "/root/problem")
    from reference import setup_inputs
    inputs = {k: np.asarray(v) for k, v in setup_inputs().items()}
    got = kernel(**inputs)
    print("out", got.shape, got.dtype)
    print(got[:2])



# revision 11
# speedup vs baseline: 1.2985x; 1.2985x over previous
"""Trainium2 Bass kernel for nn_Net_39041252721137 (supermask MLP with global
top-50% |score| masking).

Data-parallel on batch across 8 cores; replicated scores/weights. Global
top-k thresholds computed ON DEVICE per core with a count-based scheme
(all exact counts, no per-element sort):

  s1 (6.4M elements):
    A  subset (1/7) stratified estimate -> bracket [Tlo, Thi] (~±50k ranks)
    B  full stream: exact counts at Tlo/Thi + per-partition grid counts
       -> interpolated t_hat (sigma ~1.4e2 ranks) -> band3 [T3lo, T3hi]
    C  full stream: exact count below T3lo + suppress >=T3hi + 3-level
       max-pool compaction of the ~1.1k-element band into [112,112]
    R  gather band to one partition, broadcast to all, 3 stratified
       rounds of exact counting -> rank-J1 value v1 (pool collisions can
       shift the rank by a few tens; output effect ~1e-4)
  s2 (82k elements): same idea, but the band extraction is lossless
    (iterated max8/match_replace on the small resident tile) -> exact v2.

Then masked bf16 matmuls: h = relu(x @ (w1*m1).T), logits = h @ (w2*m2).T,
log_softmax fused in one pass over 64 neuron blocks.
"""
import sys

import numpy as np
import ml_dtypes

sys.path.insert(0, "/root/.axon_site")

import concourse.bass as bass
import concourse.bacc as bacc
import concourse.mybir as mybir
import concourse.tile as tile
from concourse.bass_isa import ReduceOp
from concourse.bass_utils import run_bass_kernel_spmd
from concourse.masks import make_identity

F32 = mybir.dt.float32
BF16 = mybir.dt.bfloat16
U32 = mybir.dt.uint32
AF = mybir.ActivationFunctionType
ALU = mybir.AluOpType
AX = mybir.AxisListType

N_CORES = 8
B, D_IN, N2, N_OUT = 16384, 784, 8192, 10
BS = B // N_CORES            # 2048 batch rows per core
KT, KP = 7, 112              # d_in tiled as 7 x 112 partitions
NB = N2 // 128               # 64 neuron blocks
WCOL = NB * KT * 128         # 57344 = per-partition columns of w1r/s1r
CHW = 4096                   # threshold streaming chunk width
NCH = WCOL // CHW            # 14 chunks
N1 = N2 * D_IN               # 6422528
SUBF = float(N1 // CHW)      # subset per-point extrapolation factor (1568)
J1 = N1 // 2
NS2 = N_OUT * N2             # 81920
J2 = NS2 // 2
BBS = 512
NBB = BS // BBS              # 4

MA1 = 250000.0               # s1 pass-A1 bracket margin (ranks)
MA2 = 24000.0                # s1 pass-A2 band half-width (ranks, ~5 sigma)
MB1 = 550.0                  # s1 band3 half-width (ranks)
M2A = 8000.0                 # s2 coarse bracket margin (ranks)
M2B = 350.0                  # s2 band half-width (ranks)
NR = 3                       # stratified refinement rounds (each /P width)
MX2 = 3                      # s2 max8 iterations (capacity 24/row)

_cache = {}


def _bracket(nc, pool, grid, est, jlo, jhi, Lfb, Ufb, P, tag):
    """[L, U] = (max grid pt with est<jlo, min grid pt with est>jhi),
    falling back to Lfb/Ufb. All tiles [P,1] f32; est compared to imms."""
    selL = pool.tile([P, 1], U32, tag=f"{tag}sl")
    nc.vector.tensor_scalar(selL[:], est[:], jlo, scalar2=None, op0=ALU.is_lt)
    candL = pool.tile([P, 1], F32, tag=f"{tag}cl")
    nc.vector.select(candL[:], selL[:], grid[:], Lfb[:])
    L = pool.tile([P, 1], F32, tag=f"{tag}L")
    nc.gpsimd.partition_all_reduce(L[:], candL[:], channels=P,
                                   reduce_op=ReduceOp.max)
    selU = pool.tile([P, 1], U32, tag=f"{tag}su")
    nc.vector.tensor_scalar(selU[:], est[:], jhi, scalar2=None, op0=ALU.is_gt)
    candU = pool.tile([P, 1], F32, tag=f"{tag}cu")
    nc.vector.select(candU[:], selU[:], grid[:], Ufb[:])
    nc.vector.tensor_scalar(candU[:], candU[:], -1.0, scalar2=None,
                            op0=ALU.mult)
    U = pool.tile([P, 1], F32, tag=f"{tag}U")
    nc.gpsimd.partition_all_reduce(U[:], candU[:], channels=P,
                                   reduce_op=ReduceOp.max)
    nc.vector.tensor_scalar(U[:], U[:], -1.0, scalar2=None, op0=ALU.mult)
    return L, U


def _mkgrid(nc, pool, iot, L, U, P, tag):
    """grid_p = L + p*(U-L)/P for p=1..P (t_P == U)."""
    g = pool.tile([P, 1], F32, tag=f"{tag}g")
    nc.vector.tensor_tensor(g[:], U[:], L[:], op=ALU.subtract)
    nc.vector.tensor_scalar(g[:], g[:], 1.0 / P, scalar2=None, op0=ALU.mult)
    nc.vector.tensor_tensor(g[:], iot[:], g[:], op=ALU.mult)
    nc.vector.tensor_tensor(g[:], g[:], L[:], op=ALU.add)
    return g


def _rounds_extract(nc, pool, gb_ap, scr_ap, W, P, iot, onesW, L0, U0, jp,
                    tag):
    """NR stratified rounds of exact counting on broadcast data, then
    extract the unique representable value in the final [L, U)."""
    L, U = L0, U0
    for r in range(NR):
        grid = _mkgrid(nc, pool, iot, L, U, P, tag=f"{tag}r")
        cR = pool.tile([P, 1], F32, tag=f"{tag}c")
        nc.vector.scalar_tensor_tensor(
            scr_ap, gb_ap, grid[:, :1], onesW, op0=ALU.is_lt, op1=ALU.mult,
            accum_out=cR[:])
        selL = pool.tile([P, 1], U32, tag=f"{tag}sl")
        nc.vector.tensor_tensor(selL[:], cR[:], jp[:], op=ALU.is_le)
        candL = pool.tile([P, 1], F32, tag=f"{tag}cl")
        nc.vector.select(candL[:], selL[:], grid[:], L[:])
        Ln = pool.tile([P, 1], F32, tag=f"{tag}L")
        nc.gpsimd.partition_all_reduce(Ln[:], candL[:], channels=P,
                                       reduce_op=ReduceOp.max)
        selU = pool.tile([P, 1], U32, tag=f"{tag}su")
        nc.vector.tensor_tensor(selU[:], cR[:], jp[:], op=ALU.is_gt)
        candU = pool.tile([P, 1], F32, tag=f"{tag}cu")
        nc.vector.select(candU[:], selU[:], grid[:], U[:])
        nc.vector.tensor_scalar(candU[:], candU[:], -1.0, scalar2=None,
                                op0=ALU.mult)
        Un = pool.tile([P, 1], F32, tag=f"{tag}U")
        nc.gpsimd.partition_all_reduce(Un[:], candU[:], channels=P,
                                       reduce_op=ReduceOp.max)
        nc.vector.tensor_scalar(Un[:], Un[:], -1.0, scalar2=None,
                                op0=ALU.mult)
        L, U = Ln, Un
    # v = max over values < U (the single representable value in [L, U))
    nc.vector.scalar_tensor_tensor(gb_ap, gb_ap, U[:, :1], gb_ap,
                                   op0=ALU.is_lt, op1=ALU.mult)
    v = pool.tile([P, 1], F32, tag=f"{tag}v")
    nc.vector.tensor_reduce(v[:], gb_ap, axis=AX.X, op=ALU.max)
    return v


def build_program():
    nc = bacc.Bacc("TRN2", target_bir_lowering=False, debug=False,
                   num_devices=N_CORES)

    xT = nc.declare_dram_parameter("xT", [KT, KP, BS], BF16, isOutput=False)
    w1r = nc.declare_dram_parameter("w1r", [KP, WCOL], BF16, isOutput=False)
    s1r = nc.declare_dram_parameter("s1r", [KP, WCOL], F32, isOutput=False)
    w2r = nc.declare_dram_parameter("w2r", [128, NB * N_OUT], BF16,
                                    isOutput=False)
    s2r = nc.declare_dram_parameter("s2r", [128, NB * N_OUT], F32,
                                    isOutput=False)
    out = nc.declare_dram_parameter("out", [BS, N_OUT], F32, isOutput=True)

    with tile.TileContext(nc) as tc:
        with (
            tc.tile_pool(name="state", bufs=1) as st,
            tc.tile_pool(name="small", bufs=2) as sm,
            tc.tile_pool(name="mm", bufs=3) as mmp,
            tc.tile_pool(name="hbuf", bufs=8) as hbp,
            tc.tile_pool(name="psum_h", bufs=4, space="PSUM") as psh,
            tc.tile_pool(name="psum_l", bufs=1, space="PSUM") as psl,
            tc.tile_pool(name="epi", bufs=2) as epi,
        ):
            # ---- shared constants ----
            onef = st.tile([128, 1], F32)
            nc.vector.memset(onef[:], 1.0)
            zbf16 = st.tile([128, 1], BF16)
            nc.vector.memset(zbf16[:], 0.0)
            zb = st.tile([128, 1], F32)
            nc.vector.memset(zb[:], 0.0)
            ident = st.tile([128, 128], F32)
            make_identity(nc, ident[:])
            iot112 = st.tile([KP, 1], F32)
            nc.gpsimd.iota(iot112[:], pattern=[[0, 1]], base=1,
                           channel_multiplier=1,
                           allow_small_or_imprecise_dtypes=True)
            iot128 = st.tile([128, 1], F32)
            nc.gpsimd.iota(iot128[:], pattern=[[0, 1]], base=1,
                           channel_multiplier=1,
                           allow_small_or_imprecise_dtypes=True)
            ones640 = onef[:].to_broadcast([128, NB * N_OUT])

            # x resident [112, KT*2048] bf16 (28KB/partition)
            xsb = st.tile([KP, KT * BS], BF16)
            for kt in range(KT):
                nc.sync.dma_start(xsb[:, kt * BS:(kt + 1) * BS], xT[kt])

            # ================= s2 threshold (exact) =================
            s2ctx = tc.tile_pool(name="s2p", bufs=1)
            s2p = s2ctx.__enter__()
            s2sb = s2p.tile([128, NB * N_OUT], F32)
            nc.sync.dma_start(s2sb[:], s2r[:])
            w2raw = s2p.tile([128, NB * N_OUT], BF16)
            nc.sync.dma_start(w2raw[:], w2r[:])
            a2 = s2p.tile([128, NB * N_OUT], F32)
            nc.vector.tensor_scalar(a2[:].bitcast(U32), s2sb[:].bitcast(U32),
                                    0x7FFFFFFF, scalar2=None,
                                    op0=ALU.bitwise_and)
            scr2 = s2p.tile([128, NB * N_OUT], F32)
            rm2 = sm.tile([128, 1], F32, tag="rm2")
            nc.vector.tensor_reduce(rm2[:], a2[:], axis=AX.X, op=ALU.max)
            gmax2 = st.tile([128, 1], F32)
            nc.gpsimd.partition_all_reduce(gmax2[:], rm2[:], channels=128,
                                           reduce_op=ReduceOp.max)
            gridS1 = _mkgrid(nc, sm, iot128, zb, gmax2, 128, tag="s2a")
            c2a = sm.tile([128, 1], F32, tag="c2a")
            nc.vector.scalar_tensor_tensor(
                scr2[:], a2[:], gridS1[:, :1], ones640, op0=ALU.is_lt,
                op1=ALU.mult, accum_out=c2a[:])
            chat2 = sm.tile([128, 1], F32, tag="chat2")
            nc.vector.tensor_scalar(chat2[:], c2a[:], 128.0, scalar2=None,
                                    op0=ALU.mult)
            L2, U2 = _bracket(nc, sm, gridS1, chat2, float(J2 - M2A),
                              float(J2 + M2A), zb, gmax2, 128, tag="s2b")
            # refine: exact counts below L2/U2 + grid counts
            gridS2 = _mkgrid(nc, sm, iot128, L2, U2, 128, tag="s2c")
            cL2 = sm.tile([128, 1], F32, tag="cL2")
            nc.vector.scalar_tensor_tensor(
                scr2[:], a2[:], L2[:, :1], ones640, op0=ALU.is_lt,
                op1=ALU.mult, accum_out=cL2[:])
            cU2 = sm.tile([128, 1], F32, tag="cU2")
            nc.vector.scalar_tensor_tensor(
                scr2[:], a2[:], U2[:, :1], ones640, op0=ALU.is_lt,
                op1=ALU.mult, accum_out=cU2[:])
            cg2 = sm.tile([128, 1], F32, tag="cg2")
            nc.vector.scalar_tensor_tensor(
                scr2[:], a2[:], gridS2[:, :1], ones640, op0=ALU.is_lt,
                op1=ALU.mult, accum_out=cg2[:])
            CL2 = st.tile([128, 1], F32)
            nc.gpsimd.partition_all_reduce(CL2[:], cL2[:], channels=128,
                                           reduce_op=ReduceOp.add)
            CU2 = st.tile([128, 1], F32)
            nc.gpsimd.partition_all_reduce(CU2[:], cU2[:], channels=128,
                                           reduce_op=ReduceOp.add)
            d2 = sm.tile([128, 1], F32, tag="d2")
            nc.vector.tensor_tensor(d2[:], cg2[:], cL2[:], op=ALU.subtract)
            S2 = st.tile([128, 1], F32)
            nc.gpsimd.partition_all_reduce(S2[:], d2[:], channels=128,
                                           reduce_op=ReduceOp.add)
            # t_hat = midW + (J2 - (CL2 + S2)) * (U2-L2)/(CU2-CL2)
            wid2 = sm.tile([128, 1], F32, tag="wid2")
            nc.vector.tensor_tensor(wid2[:], U2[:], L2[:], op=ALU.subtract)
            den2 = sm.tile([128, 1], F32, tag="den2")
            nc.vector.tensor_tensor(den2[:], CU2[:], CL2[:], op=ALU.subtract)
            rho2i = sm.tile([128, 1], F32, tag="rho2i")
            nc.vector.reciprocal(rho2i[:], den2[:])
            nc.vector.tensor_tensor(rho2i[:], rho2i[:], wid2[:], op=ALU.mult)
            mid2 = sm.tile([128, 1], F32, tag="mid2")
            nc.vector.tensor_scalar(mid2[:], wid2[:], 129.0 / 256.0,
                                    scalar2=None, op0=ALU.mult)
            nc.vector.tensor_tensor(mid2[:], mid2[:], L2[:], op=ALU.add)
            rr2 = sm.tile([128, 1], F32, tag="rr2")
            nc.vector.tensor_tensor(rr2[:], CL2[:], S2[:], op=ALU.add)
            nc.vector.tensor_scalar(rr2[:], rr2[:], -1.0, scalar2=float(J2),
                                    op0=ALU.mult, op1=ALU.add)
            that2 = sm.tile([128, 1], F32, tag="that2")
            nc.vector.tensor_tensor(that2[:], rr2[:], rho2i[:], op=ALU.mult)
            nc.vector.tensor_tensor(that2[:], that2[:], mid2[:], op=ALU.add)
            mrg2 = sm.tile([128, 1], F32, tag="mrg2")
            nc.vector.tensor_scalar(mrg2[:], rho2i[:], M2B, scalar2=None,
                                    op0=ALU.mult)
            T2lo = st.tile([128, 1], F32)
            nc.vector.tensor_tensor(T2lo[:], that2[:], mrg2[:],
                                    op=ALU.subtract)
            T2hi = st.tile([128, 1], F32)
            nc.vector.tensor_tensor(T2hi[:], that2[:], mrg2[:], op=ALU.add)
            # exact count below T2lo
            cb2 = sm.tile([128, 1], F32, tag="cb2")
            nc.vector.scalar_tensor_tensor(
                scr2[:], a2[:], T2lo[:, :1], ones640, op0=ALU.is_lt,
                op1=ALU.mult, accum_out=cb2[:])
            CB2 = st.tile([128, 1], F32)
            nc.gpsimd.partition_all_reduce(CB2[:], cb2[:], channels=128,
                                           reduce_op=ReduceOp.add)
            # band extraction (lossless): z2 = (a2 < T2hi)*a2, iterated max8
            z2 = s2p.tile([128, NB * N_OUT], F32)
            nc.vector.scalar_tensor_tensor(z2[:], a2[:], T2hi[:, :1], a2[:],
                                           op0=ALU.is_lt, op1=ALU.mult)
            B2s = s2p.tile([128, MX2 * 8], F32)
            src = z2
            for i in range(MX2):
                mx = B2s[:, i * 8:(i + 1) * 8]
                nc.vector.max(out=mx, in_=src[:])
                if i < MX2 - 1:
                    nxt = s2p.tile([128, NB * N_OUT], F32,
                                   name=f"s2mr{i}", tag=f"s2mr{i % 2}")
                    nc.vector.match_replace(out=nxt[:], in_to_replace=mx,
                                            in_values=src[:], imm_value=-1.0)
                    src = nxt
            # gather band to partition 0, broadcast, refine rounds
            g2 = s2p.tile([1, 128 * MX2 * 8], F32)
            nc.sync.dma_start(g2[:], B2s[:])
            gb2 = s2p.tile([128, 128 * MX2 * 8], F32)
            nc.gpsimd.partition_broadcast(gb2[:], g2[:], channels=128)
            scrb2 = s2p.tile([128, 128 * MX2 * 8], BF16)
            onesg2 = onef[:].to_broadcast([128, 128 * MX2 * 8])
            grb2 = sm.tile([128, 1], F32, tag="grb2")
            nc.vector.scalar_tensor_tensor(
                scrb2[:], gb2[:], T2lo[:, :1], onesg2, op0=ALU.is_lt,
                op1=ALU.mult, accum_out=grb2[:])
            j2p = sm.tile([128, 1], F32, tag="j2p")
            nc.vector.tensor_scalar(j2p[:], CB2[:], -1.0, scalar2=float(J2),
                                    op0=ALU.mult, op1=ALU.add)
            nc.vector.tensor_tensor(j2p[:], j2p[:], grb2[:], op=ALU.add)
            v2 = _rounds_extract(nc, sm, gb2[:], scrb2[:], 128 * MX2 * 8, 128,
                                 iot128, onesg2, T2lo, T2hi, j2p, tag="s2r")
            # masked w2 (a2 = |s2| already computed)
            pr2 = s2p.tile([128, NB * N_OUT], U32)
            nc.vector.tensor_scalar(pr2[:], a2[:], v2[:, :1], scalar2=None,
                                    op0=ALU.is_lt)
            w2m = st.tile([128, NB * N_OUT], BF16)
            nc.vector.select(w2m[:], pr2[:],
                             zbf16[:].to_broadcast([128, NB * N_OUT]),
                             w2raw[:])
            s2ctx.__exit__(None, None, None)

            # ================= s1 threshold =================
            ones_ch = onef[:KP].to_broadcast([KP, CHW])
            with (
                tc.tile_pool(name="zp", bufs=1) as zpp,
            ):
                with (
                    tc.tile_pool(name="strm", bufs=2) as strm,
                    tc.tile_pool(name="absp", bufs=2) as absp,
                ):
                    # ---- pass A: subset (first chunk = 8192 cols) ----
                    rawA = strm.tile([KP, CHW], F32, tag="raw", name="rawA")
                    nc.sync.dma_start(rawA[:], s1r[:, 0:CHW])
                    aA = absp.tile([KP, CHW], F32, tag="abs", name="aA")
                    nc.scalar.activation(aA[:], rawA[:], AF.Abs, bias=0.0,
                                         scale=1.0)
                    rmax = sm.tile([KP, 1], F32, tag="rmax")
                    nc.vector.tensor_reduce(rmax[:], aA[:], axis=AX.X,
                                            op=ALU.max)
                    gmax1 = st.tile([KP, 1], F32)
                    nc.gpsimd.partition_all_reduce(gmax1[:], rmax[:],
                                                   channels=KP,
                                                   reduce_op=ReduceOp.max)
                    gridA1 = _mkgrid(nc, sm, iot112, zb[:KP], gmax1, KP,
                                     tag="a1")
                    cA1 = sm.tile([KP, 1], F32, tag="cA1")
                    nc.vector.scalar_tensor_tensor(
                        rawA[:], aA[:], gridA1[:, :1], ones_ch, op0=ALU.is_lt,
                        op1=ALU.mult, accum_out=cA1[:])
                    chatA = sm.tile([KP, 1], F32, tag="chatA")
                    nc.vector.tensor_scalar(chatA[:], cA1[:], SUBF,
                                            scalar2=None, op0=ALU.mult)
                    LA, UA = _bracket(nc, sm, gridA1, chatA, float(J1 - MA1),
                                      float(J1 + MA1), zb[:KP], gmax1, KP,
                                      tag="aB")
                    # ---- pass A2: anchored S-sum interpolation on subset ----
                    # t_hat = midA + (J1 - 7*(CLa + SdA)) * widA/(7*(CUa-CLa))
                    gridA2 = _mkgrid(nc, sm, iot112, LA, UA, KP, tag="a2")
                    cLa = sm.tile([KP, 1], F32, tag="cLa")
                    nc.vector.scalar_tensor_tensor(
                        rawA[:], aA[:], LA[:, :1], ones_ch, op0=ALU.is_lt,
                        op1=ALU.mult, accum_out=cLa[:])
                    cUa = sm.tile([KP, 1], F32, tag="cUa")
                    nc.vector.scalar_tensor_tensor(
                        rawA[:], aA[:], UA[:, :1], ones_ch, op0=ALU.is_lt,
                        op1=ALU.mult, accum_out=cUa[:])
                    cga = sm.tile([KP, 1], F32, tag="cga")
                    nc.vector.scalar_tensor_tensor(
                        rawA[:], aA[:], gridA2[:, :1], ones_ch, op0=ALU.is_lt,
                        op1=ALU.mult, accum_out=cga[:])
                    CLa = st.tile([KP, 1], F32)
                    nc.gpsimd.partition_all_reduce(CLa[:], cLa[:], channels=KP,
                                                   reduce_op=ReduceOp.add)
                    CUa = st.tile([KP, 1], F32)
                    nc.gpsimd.partition_all_reduce(CUa[:], cUa[:], channels=KP,
                                                   reduce_op=ReduceOp.add)
                    dA = sm.tile([KP, 1], F32, tag="dA")
                    nc.vector.tensor_tensor(dA[:], cga[:], cLa[:],
                                            op=ALU.subtract)
                    SdA = st.tile([KP, 1], F32)
                    nc.gpsimd.partition_all_reduce(SdA[:], dA[:], channels=KP,
                                                   reduce_op=ReduceOp.add)
                    widA = sm.tile([KP, 1], F32, tag="widA")
                    nc.vector.tensor_tensor(widA[:], UA[:], LA[:],
                                            op=ALU.subtract)
                    denA = sm.tile([KP, 1], F32, tag="denA")
                    nc.vector.tensor_tensor(denA[:], CUa[:], CLa[:],
                                            op=ALU.subtract)
                    nc.vector.tensor_scalar(denA[:], denA[:], SUBF / 112.0,
                                            scalar2=None, op0=ALU.mult)
                    rhoAi = sm.tile([KP, 1], F32, tag="rhoAi")
                    nc.vector.reciprocal(rhoAi[:], denA[:])
                    nc.vector.tensor_tensor(rhoAi[:], rhoAi[:], widA[:],
                                            op=ALU.mult)
                    midA = sm.tile([KP, 1], F32, tag="midA")
                    nc.vector.tensor_scalar(midA[:], widA[:], 113.0 / 224.0,
                                            scalar2=None, op0=ALU.mult)
                    nc.vector.tensor_tensor(midA[:], midA[:], LA[:],
                                            op=ALU.add)
                    rrA = sm.tile([KP, 1], F32, tag="rrA")
                    nc.vector.tensor_tensor(rrA[:], CLa[:], SdA[:],
                                            op=ALU.add)
                    nc.vector.tensor_scalar(rrA[:], rrA[:], -SUBF / 112.0,
                                            scalar2=float(J1), op0=ALU.mult,
                                            op1=ALU.add)
                    thatA = sm.tile([KP, 1], F32, tag="thatA")
                    nc.vector.tensor_tensor(thatA[:], rrA[:], rhoAi[:],
                                            op=ALU.mult)
                    nc.vector.tensor_tensor(thatA[:], thatA[:], midA[:],
                                            op=ALU.add)
                    mrgA = sm.tile([KP, 1], F32, tag="mrgA")
                    nc.vector.tensor_scalar(mrgA[:], rhoAi[:], MA2,
                                            scalar2=None, op0=ALU.mult)
                    Tlo = st.tile([KP, 1], F32)
                    nc.vector.tensor_tensor(Tlo[:], thatA[:], mrgA[:],
                                            op=ALU.subtract)
                    Thi = st.tile([KP, 1], F32)
                    nc.vector.tensor_tensor(Thi[:], thatA[:], mrgA[:],
                                            op=ALU.add)

                    # ---- pass B: full stream, exact ends + grid ----
                    gridB = _mkgrid(nc, sm, iot112, Tlo, Thi, KP, tag="b")
                    bTlo = st.tile([KP, 1], F32)
                    nc.vector.memset(bTlo[:], 0.0)
                    bThi = st.tile([KP, 1], F32)
                    nc.vector.memset(bThi[:], 0.0)
                    bGrd = st.tile([KP, 1], F32)
                    nc.vector.memset(bGrd[:], 0.0)
                    for kt in range(NCH):
                        raw = strm.tile([KP, CHW], F32, tag="raw",
                                        name=f"rawB{kt}")
                        nc.sync.dma_start(raw[:],
                                          s1r[:, kt * CHW:(kt + 1) * CHW])
                        a = absp.tile([KP, CHW], F32, tag="abs",
                                      name=f"aB{kt}")
                        nc.scalar.activation(a[:], raw[:], AF.Abs, bias=0.0,
                                             scale=1.0)
                        for thr, acc in ((Tlo, bTlo), (Thi, bThi),
                                         (gridB, bGrd)):
                            ck = sm.tile([KP, 1], F32, tag="ckB")
                            nc.vector.scalar_tensor_tensor(
                                raw[:], a[:], thr[:, :1], ones_ch,
                                op0=ALU.is_lt, op1=ALU.mult, accum_out=ck[:])
                            nc.vector.tensor_tensor(acc[:], acc[:], ck[:],
                                                    op=ALU.add)
                    Clo = st.tile([KP, 1], F32)
                    nc.gpsimd.partition_all_reduce(Clo[:], bTlo[:],
                                                   channels=KP,
                                                   reduce_op=ReduceOp.add)
                    Chi = st.tile([KP, 1], F32)
                    nc.gpsimd.partition_all_reduce(Chi[:], bThi[:],
                                                   channels=KP,
                                                   reduce_op=ReduceOp.add)
                    dB = sm.tile([KP, 1], F32, tag="dB")
                    nc.vector.tensor_tensor(dB[:], bGrd[:], bTlo[:],
                                            op=ALU.subtract)
                    SB = st.tile([KP, 1], F32)
                    nc.gpsimd.partition_all_reduce(SB[:], dB[:], channels=KP,
                                                   reduce_op=ReduceOp.add)
                    widB = sm.tile([KP, 1], F32, tag="widB")
                    nc.vector.tensor_tensor(widB[:], Thi[:], Tlo[:],
                                            op=ALU.subtract)
                    denB = sm.tile([KP, 1], F32, tag="denB")
                    nc.vector.tensor_tensor(denB[:], Chi[:], Clo[:],
                                            op=ALU.subtract)
                    rhoBi = sm.tile([KP, 1], F32, tag="rhoBi")
                    nc.vector.reciprocal(rhoBi[:], denB[:])
                    nc.vector.tensor_tensor(rhoBi[:], rhoBi[:], widB[:],
                                            op=ALU.mult)
                    midB = sm.tile([KP, 1], F32, tag="midB")
                    nc.vector.tensor_scalar(midB[:], widB[:], 113.0 / 224.0,
                                            scalar2=None, op0=ALU.mult)
                    nc.vector.tensor_tensor(midB[:], midB[:], Tlo[:],
                                            op=ALU.add)
                    rrB = sm.tile([KP, 1], F32, tag="rrB")
                    nc.vector.tensor_tensor(rrB[:], Clo[:], SB[:], op=ALU.add)
                    nc.vector.tensor_scalar(rrB[:], rrB[:], -1.0,
                                            scalar2=float(J1), op0=ALU.mult,
                                            op1=ALU.add)
                    thatB = sm.tile([KP, 1], F32, tag="thatB")
                    nc.vector.tensor_tensor(thatB[:], rrB[:], rhoBi[:],
                                            op=ALU.mult)
                    nc.vector.tensor_tensor(thatB[:], thatB[:], midB[:],
                                            op=ALU.add)
                    mrgB = sm.tile([KP, 1], F32, tag="mrgB")
                    nc.vector.tensor_scalar(mrgB[:], rhoBi[:], MB1,
                                            scalar2=None, op0=ALU.mult)
                    T3lo = st.tile([KP, 1], F32)
                    nc.vector.tensor_tensor(T3lo[:], thatB[:], mrgB[:],
                                            op=ALU.subtract)
                    T3hi = st.tile([KP, 1], F32)
                    nc.vector.tensor_tensor(T3hi[:], thatB[:], mrgB[:],
                                            op=ALU.add)

                    # ---- pass C: exact base count + band pool-compaction ----
                    zp = zpp.tile([KP, NCH * (CHW // 16)], F32)
                    cC = st.tile([KP, 1], F32)
                    nc.vector.memset(cC[:], 0.0)
                    for kt in range(NCH):
                        raw = strm.tile([KP, CHW], F32, tag="raw",
                                        name=f"rawC{kt}")
                        nc.sync.dma_start(raw[:],
                                          s1r[:, kt * CHW:(kt + 1) * CHW])
                        a = absp.tile([KP, CHW], F32, tag="abs",
                                      name=f"aC{kt}")
                        nc.scalar.activation(a[:], raw[:], AF.Abs, bias=0.0,
                                             scale=1.0)
                        ck = sm.tile([KP, 1], F32, tag="ckC")
                        nc.vector.scalar_tensor_tensor(
                            raw[:], a[:], T3lo[:, :1], ones_ch, op0=ALU.is_lt,
                            op1=ALU.mult, accum_out=ck[:])
                        nc.vector.tensor_tensor(cC[:], cC[:], ck[:],
                                                op=ALU.add)
                        # z = (a < T3hi) * a  (suppress above-band), written
                        # over the raw tile, then 16:1 max-pool into zp
                        nc.vector.scalar_tensor_tensor(
                            raw[:], a[:], T3hi[:, :1], a[:], op0=ALU.is_lt,
                            op1=ALU.mult)
                        nc.vector.tensor_reduce(
                            zp[:, kt * (CHW // 16):(kt + 1) * (CHW // 16)],
                            raw[:].rearrange("p (g k) -> p g k", k=16),
                            axis=AX.X, op=ALU.max)
                    C3 = st.tile([KP, 1], F32)
                    nc.gpsimd.partition_all_reduce(C3[:], cC[:], channels=KP,
                                                   reduce_op=ReduceOp.add)
                # strm/absp freed here; pool down to [112, 112] and gather
                zq = zpp.tile([KP, 224], F32)
                nc.vector.tensor_reduce(
                    zq[:], zp[:].rearrange("p (g k) -> p g k", k=16),
                    axis=AX.X, op=ALU.max)
                zr = zpp.tile([KP, 112], F32)
                nc.vector.tensor_reduce(
                    zr[:], zq[:].rearrange("p (g k) -> p g k", k=2),
                    axis=AX.X, op=ALU.max)
                with tc.tile_pool(name="rounds", bufs=1) as rdp:
                    WB = KP * 112  # 12544
                    gb = rdp.tile([KP, WB], F32)
                    nc.sync.dma_start(gb[0:1, :], zr[:])
                    nc.gpsimd.partition_broadcast(gb[:], gb[0:1, :],
                                                  channels=KP)
                    scrR = rdp.tile([KP, WB], BF16)
                    onesW = onef[:KP].to_broadcast([KP, WB])
                    grb = sm.tile([KP, 1], F32, tag="grb")
                    nc.vector.scalar_tensor_tensor(
                        scrR[:], gb[:], T3lo[:, :1], onesW, op0=ALU.is_lt,
                        op1=ALU.mult, accum_out=grb[:])
                    j1p = sm.tile([KP, 1], F32, tag="j1p")
                    nc.vector.tensor_scalar(j1p[:], C3[:], -1.0,
                                            scalar2=float(J1), op0=ALU.mult,
                                            op1=ALU.add)
                    nc.vector.tensor_tensor(j1p[:], j1p[:], grb[:],
                                            op=ALU.add)
                    v1 = _rounds_extract(nc, sm, gb[:], scrR[:], WB, KP,
                                         iot112, onesW, T3lo, T3hi, j1p,
                                         tag="s1r")
                    v1s = st.tile([KP, 1], F32)
                    nc.vector.tensor_copy(v1s[:], v1[:])

            # ================= matmul pipeline =================
            lgps = [psl.tile([N_OUT, BBS], F32, tag=f"lg{bb}", name=f"lg{bb}")
                    for bb in range(NBB)]
            for nb in range(NB):
                w1b = mmp.tile([KP, KT * 128], BF16, tag="w1b")
                nc.sync.dma_start(w1b[:],
                                  w1r[:, nb * KT * 128:(nb + 1) * KT * 128])
                s1b = mmp.tile([KP, KT * 128], F32, tag="s1b")
                nc.sync.dma_start(s1b[:],
                                  s1r[:, nb * KT * 128:(nb + 1) * KT * 128])
                nc.vector.tensor_scalar(s1b[:].bitcast(U32),
                                        s1b[:].bitcast(U32), 0x7FFFFFFF,
                                        scalar2=None, op0=ALU.bitwise_and)
                nc.vector.tensor_scalar(s1b[:].bitcast(U32), s1b[:],
                                        v1s[:, :1], scalar2=None,
                                        op0=ALU.is_lt)
                w1m = mmp.tile([KP, KT * 128], BF16, tag="w1m")
                nc.vector.select(w1m[:], s1b[:].bitcast(U32),
                                 zbf16[:KP].to_broadcast([KP, KT * 128]),
                                 w1b[:])
                hts = []
                for bb in range(NBB):
                    ph = psh.tile([128, BBS], F32, tag="ph")
                    for kt in range(KT):
                        nc.tensor.matmul(
                            ph[:], w1m[:, kt * 128:(kt + 1) * 128],
                            xsb[:, kt * BS + bb * BBS:
                                kt * BS + (bb + 1) * BBS],
                            start=(kt == 0), stop=(kt == KT - 1))
                    ht = hbp.tile([128, BBS], BF16, tag="ht")
                    nc.scalar.activation(ht[:], ph[:], AF.Relu, bias=0.0,
                                         scale=1.0)
                    hts.append(ht)
                w2s = w2m[:, nb * N_OUT:(nb + 1) * N_OUT]
                for bb in range(NBB):
                    nc.tensor.matmul(lgps[bb][:], w2s, hts[bb][:],
                                     start=(nb == 0), stop=(nb == NB - 1),
                                     skip_group_check=True)

            # ================= epilogue: log_softmax =================
            lga = epi.tile([128, 16 * N_OUT], F32, tag="lga")
            for bb in range(NBB):
                lg = epi.tile([N_OUT, BBS], F32, tag="lgc")
                nc.vector.tensor_copy(lg[:], lgps[bb][:])
                for c in range(BBS // 128):
                    g = bb * 4 + c
                    pt = psh.tile([128, BBS], F32, tag="ph")
                    nc.tensor.transpose(pt[:, :N_OUT],
                                        lg[:, c * 128:(c + 1) * 128],
                                        ident[:N_OUT, :N_OUT])
                    nc.vector.tensor_copy(lga[:, g * N_OUT:(g + 1) * N_OUT],
                                          pt[:, :N_OUT])
            lga3 = lga[:].rearrange("p (g k) -> p g k", k=N_OUT)
            mx = epi.tile([128, 16], F32, tag="mx")
            nc.vector.tensor_reduce(mx[:], lga3, axis=AX.X, op=ALU.max)
            mxb = mx[:].unsqueeze(2).to_broadcast([128, 16, N_OUT])
            nc.vector.tensor_tensor(lga3, lga3, mxb, op=ALU.subtract)
            ex = epi.tile([128, 16 * N_OUT], F32, tag="ex")
            nc.scalar.activation(ex[:], lga[:], AF.Exp, bias=0.0, scale=1.0)
            se = epi.tile([128, 16], F32, tag="se")
            nc.vector.tensor_reduce(se[:],
                                    ex[:].rearrange("p (g k) -> p g k",
                                                    k=N_OUT),
                                    axis=AX.X, op=ALU.add)
            ls = epi.tile([128, 16], F32, tag="ls")
            nc.scalar.activation(ls[:], se[:], AF.Ln, bias=zb[:, :1],
                                 scale=1.0)
            lsb = ls[:].unsqueeze(2).to_broadcast([128, 16, N_OUT])
            nc.vector.tensor_tensor(lga3, lga3, lsb, op=ALU.subtract)
            for g in range(16):
                nc.sync.dma_start(out[g * 128:(g + 1) * 128, :],
                                  lga[:, g * N_OUT:(g + 1) * N_OUT])
    nc.compile()
    return nc


def _prep_inputs(x, w1, s1, w2, s2):
    bf = ml_dtypes.bfloat16
    w1r = np.ascontiguousarray(
        w1.reshape(NB, 128, KT, KP).transpose(3, 0, 2, 1).reshape(KP, WCOL)
    ).astype(bf)
    s1r = np.ascontiguousarray(
        s1.reshape(NB, 128, KT, KP).transpose(3, 0, 2, 1).reshape(KP, WCOL)
    ).astype(np.float32)
    w2r = np.ascontiguousarray(
        w2.T.reshape(NB, 128, N_OUT).transpose(1, 0, 2).reshape(128,
                                                                NB * N_OUT)
    ).astype(bf)
    s2r = np.ascontiguousarray(
        s2.T.reshape(NB, 128, N_OUT).transpose(1, 0, 2).reshape(128,
                                                                NB * N_OUT)
    ).astype(np.float32)
    in_maps = []
    for cid in range(N_CORES):
        xc = np.ascontiguousarray(
            x[cid * BS:(cid + 1) * BS].T).reshape(KT, KP, BS).astype(bf)
        in_maps.append({"xT": xc, "w1r": w1r, "s1r": s1r,
                        "w2r": w2r, "s2r": s2r})
    return in_maps


def kernel(x, w1, s1, w2, s2):
    x = np.asarray(x); w1 = np.asarray(w1); s1 = np.asarray(s1)
    w2 = np.asarray(w2); s2 = np.asarray(s2)
    if "nc" not in _cache:
        _cache["nc"] = build_program()
    nc = _cache["nc"]
    in_maps = _prep_inputs(x, w1, s1, w2, s2)
    res = run_bass_kernel_spmd(nc, in_maps, list(range(N_CORES)))
    return np.concatenate([res.results[c]["out"] for c in range(N_CORES)],
                          axis=0)


if __name__ == "__main__":
    sys.path.insert(0, "/root/problem")
    from reference import setup_inputs
    inputs = {k: np.asarray(v) for k, v in setup_inputs().items()}
    got = kernel(**inputs)
    print("out", got.shape, got.dtype)
    print(got[:2])


# revision 14
# speedup vs baseline: 2.0527x; 1.5808x over previous
"""Trainium2 Bass kernel for nn_Net_39041252721137 (supermask MLP with global
top-50% |score| masking).

Data-parallel on batch across 8 cores; replicated scores/weights. Global
top-k thresholds computed ON DEVICE per core with a count-based scheme
(all exact counts, no per-element sort):

  s1 (6.4M elements):
    A  subset (1/7) stratified estimate -> bracket [Tlo, Thi] (~±50k ranks)
    B  full stream: exact counts at Tlo/Thi + per-partition grid counts
       -> interpolated t_hat (sigma ~1.4e2 ranks) -> band3 [T3lo, T3hi]
    C  full stream: exact count below T3lo + suppress >=T3hi + 3-level
       max-pool compaction of the ~1.1k-element band into [112,112]
    R  gather band to one partition, broadcast to all, 3 stratified
       rounds of exact counting -> rank-J1 value v1 (pool collisions can
       shift the rank by a few tens; output effect ~1e-4)
  s2 (82k elements): same idea, but the band extraction is lossless
    (iterated max8/match_replace on the small resident tile) -> exact v2.

Then masked bf16 matmuls: h = relu(x @ (w1*m1).T), logits = h @ (w2*m2).T,
log_softmax fused in one pass over 64 neuron blocks.
"""
import sys

import numpy as np
import ml_dtypes

sys.path.insert(0, "/root/.axon_site")

import concourse.bass as bass
import concourse.bacc as bacc
import concourse.mybir as mybir
import concourse.tile as tile
from concourse.bass_isa import ReduceOp
from concourse.bass_utils import run_bass_kernel_spmd
from concourse.masks import make_identity

F32 = mybir.dt.float32
BF16 = mybir.dt.bfloat16
U32 = mybir.dt.uint32
AF = mybir.ActivationFunctionType
ALU = mybir.AluOpType
AX = mybir.AxisListType

N_CORES = 8
B, D_IN, N2, N_OUT = 16384, 784, 8192, 10
BS = B // N_CORES            # 2048 batch rows per core
KT, KP = 7, 112              # d_in tiled as 7 x 112 partitions
NB = N2 // 128               # 64 neuron blocks
WCOL = NB * KT * 128         # 57344 = per-partition columns of w1r/s1r
CHW = 4096                   # threshold streaming chunk width
NCH = WCOL // CHW            # 14 chunks
SH = WCOL // N_CORES         # 7168 shard columns per core
N1 = N2 * D_IN               # 6422528
SUBF = float(N1 // CHW)      # subset per-point extrapolation factor (1568)
J1 = N1 // 2
NS2 = N_OUT * N2             # 81920
J2 = NS2 // 2
BBS = 512
NBB = BS // BBS              # 4

MA1 = 250000.0               # s1 pass-A1 bracket margin (ranks)
MA2 = 24000.0                # s1 pass-A2 band half-width (ranks, ~5 sigma)
MB1 = 520.0                  # s1 band3 half-width (ranks)
M2A = 8000.0                 # s2 coarse bracket margin (ranks)
M2B = 350.0                  # s2 band half-width (ranks)
NR = 3                       # stratified refinement rounds (each /P width)
MX2 = 3                      # s2 max8 iterations (capacity 24/row)

_cache = {}


def _bracket(nc, pool, grid, est, jlo, jhi, Lfb, Ufb, P, tag):
    """[L, U] = (max grid pt with est<jlo, min grid pt with est>jhi),
    falling back to Lfb/Ufb. All tiles [P,1] f32; est compared to imms."""
    selL = pool.tile([P, 1], U32, tag=f"{tag}sl")
    nc.vector.tensor_scalar(selL[:], est[:], jlo, scalar2=None, op0=ALU.is_lt)
    candL = pool.tile([P, 1], F32, tag=f"{tag}cl")
    nc.vector.select(candL[:], selL[:], grid[:], Lfb[:])
    L = pool.tile([P, 1], F32, tag=f"{tag}L")
    nc.gpsimd.partition_all_reduce(L[:], candL[:], channels=P,
                                   reduce_op=ReduceOp.max)
    selU = pool.tile([P, 1], U32, tag=f"{tag}su")
    nc.vector.tensor_scalar(selU[:], est[:], jhi, scalar2=None, op0=ALU.is_gt)
    candU = pool.tile([P, 1], F32, tag=f"{tag}cu")
    nc.vector.select(candU[:], selU[:], grid[:], Ufb[:])
    nc.vector.tensor_scalar(candU[:], candU[:], -1.0, scalar2=None,
                            op0=ALU.mult)
    U = pool.tile([P, 1], F32, tag=f"{tag}U")
    nc.gpsimd.partition_all_reduce(U[:], candU[:], channels=P,
                                   reduce_op=ReduceOp.max)
    nc.vector.tensor_scalar(U[:], U[:], -1.0, scalar2=None, op0=ALU.mult)
    return L, U


def _mkgrid(nc, pool, iot, L, U, P, tag):
    """grid_p = L + p*(U-L)/P for p=1..P (t_P == U)."""
    g = pool.tile([P, 1], F32, tag=f"{tag}g")
    nc.vector.tensor_tensor(g[:], U[:], L[:], op=ALU.subtract)
    nc.vector.tensor_scalar(g[:], g[:], 1.0 / P, scalar2=None, op0=ALU.mult)
    nc.vector.tensor_tensor(g[:], iot[:], g[:], op=ALU.mult)
    nc.vector.tensor_tensor(g[:], g[:], L[:], op=ALU.add)
    return g


def _interp_band(nc, pool, st, cloAP, chiAP, cgAP, L, U, P, scale, margin,
                 tag):
    """Anchored S-sum interpolation: counts (already summed over partitions
    and cores) at L, U, and the P-point grid spanning [L, U]; returns band
    [lo, hi] = t_hat -+ margin ranks around the rank-J1 interpolant.
    scale converts counts to full-data ranks."""
    wid = pool.tile([P, 1], F32, tag=f"{tag}w")
    nc.vector.tensor_tensor(wid[:], U[:], L[:], op=ALU.subtract)
    den = pool.tile([P, 1], F32, tag=f"{tag}d")
    nc.vector.tensor_tensor(den[:], chiAP, cloAP, op=ALU.subtract)
    nc.vector.tensor_scalar(den[:], den[:], scale, scalar2=None, op0=ALU.mult)
    rhoi = pool.tile([P, 1], F32, tag=f"{tag}ri")
    nc.vector.reciprocal(rhoi[:], den[:])
    nc.vector.tensor_tensor(rhoi[:], rhoi[:], wid[:], op=ALU.mult)
    mid = pool.tile([P, 1], F32, tag=f"{tag}m")
    nc.vector.tensor_scalar(mid[:], wid[:], (P + 1.0) / (2.0 * P),
                            scalar2=None, op0=ALU.mult)
    nc.vector.tensor_tensor(mid[:], mid[:], L[:], op=ALU.add)
    rr = pool.tile([P, 1], F32, tag=f"{tag}rr")
    nc.vector.tensor_scalar(rr[:], cgAP, -scale, scalar2=float(J1),
                            op0=ALU.mult, op1=ALU.add)
    that = pool.tile([P, 1], F32, tag=f"{tag}t")
    nc.vector.tensor_tensor(that[:], rr[:], rhoi[:], op=ALU.mult)
    nc.vector.tensor_tensor(that[:], that[:], mid[:], op=ALU.add)
    mrg = pool.tile([P, 1], F32, tag=f"{tag}mg")
    nc.vector.tensor_scalar(mrg[:], rhoi[:], margin, scalar2=None,
                            op0=ALU.mult)
    lo = st.tile([P, 1], F32, name=f"{tag}lo")
    nc.vector.tensor_tensor(lo[:], that[:], mrg[:], op=ALU.subtract)
    hi = st.tile([P, 1], F32, name=f"{tag}hi")
    nc.vector.tensor_tensor(hi[:], that[:], mrg[:], op=ALU.add)
    return lo, hi


def _rounds_extract(nc, pool, gb_ap, scr_ap, W, P, iot, onesW, L0, U0, jp,
                    n_rounds, tag):
    """n_rounds stratified rounds of exact counting on broadcast data, then
    extract the unique representable value in the final [L, U)."""
    L, U = L0, U0
    for r in range(n_rounds):
        grid = _mkgrid(nc, pool, iot, L, U, P, tag=f"{tag}r")
        cR = pool.tile([P, 1], F32, tag=f"{tag}c")
        nc.vector.scalar_tensor_tensor(
            scr_ap, gb_ap, grid[:, :1], onesW, op0=ALU.is_lt, op1=ALU.mult,
            accum_out=cR[:])
        selL = pool.tile([P, 1], U32, tag=f"{tag}sl")
        nc.vector.tensor_tensor(selL[:], cR[:], jp[:], op=ALU.is_le)
        candL = pool.tile([P, 1], F32, tag=f"{tag}cl")
        nc.vector.select(candL[:], selL[:], grid[:], L[:])
        Ln = pool.tile([P, 1], F32, tag=f"{tag}L")
        nc.gpsimd.partition_all_reduce(Ln[:], candL[:], channels=P,
                                       reduce_op=ReduceOp.max)
        selU = pool.tile([P, 1], U32, tag=f"{tag}su")
        nc.vector.tensor_tensor(selU[:], cR[:], jp[:], op=ALU.is_gt)
        candU = pool.tile([P, 1], F32, tag=f"{tag}cu")
        nc.vector.select(candU[:], selU[:], grid[:], U[:])
        nc.vector.tensor_scalar(candU[:], candU[:], -1.0, scalar2=None,
                                op0=ALU.mult)
        Un = pool.tile([P, 1], F32, tag=f"{tag}U")
        nc.gpsimd.partition_all_reduce(Un[:], candU[:], channels=P,
                                       reduce_op=ReduceOp.max)
        nc.vector.tensor_scalar(Un[:], Un[:], -1.0, scalar2=None,
                                op0=ALU.mult)
        L, U = Ln, Un
    # v = max over values < U (the single representable value in [L, U))
    nc.vector.scalar_tensor_tensor(gb_ap, gb_ap, U[:, :1], gb_ap,
                                   op0=ALU.is_lt, op1=ALU.mult)
    v = pool.tile([P, 1], F32, tag=f"{tag}v")
    nc.vector.tensor_reduce(v[:], gb_ap, axis=AX.X, op=ALU.max)
    return v


def build_program():
    nc = bacc.Bacc("TRN2", target_bir_lowering=False, debug=False,
                   num_devices=N_CORES)

    xT = nc.declare_dram_parameter("xT", [KT, KP, BS], BF16, isOutput=False)
    w1r = nc.declare_dram_parameter("w1r", [KP, WCOL], BF16, isOutput=False)
    s1r = nc.declare_dram_parameter("s1r", [KP, WCOL], F32, isOutput=False)
    s1sh = nc.declare_dram_parameter("s1sh", [KP, SH], F32, isOutput=False)
    w2r = nc.declare_dram_parameter("w2r", [128, NB * N_OUT], BF16,
                                    isOutput=False)
    s2r = nc.declare_dram_parameter("s2r", [128, NB * N_OUT], F32,
                                    isOutput=False)
    out = nc.declare_dram_parameter("out", [BS, N_OUT], F32, isOutput=True)

    with tile.TileContext(nc) as tc:
        with (
            tc.tile_pool(name="state", bufs=1) as st,
            tc.tile_pool(name="small", bufs=2) as sm,
            tc.tile_pool(name="mm", bufs=3) as mmp,
            tc.tile_pool(name="hbuf", bufs=8) as hbp,
            tc.tile_pool(name="psum_h", bufs=4, space="PSUM") as psh,
            tc.tile_pool(name="psum_l", bufs=1, space="PSUM") as psl,
            tc.tile_pool(name="epi", bufs=2) as epi,
        ):
            # ---- shared constants ----
            onef = st.tile([128, 1], F32)
            nc.vector.memset(onef[:], 1.0)
            zbf16 = st.tile([128, 1], BF16)
            nc.vector.memset(zbf16[:], 0.0)
            zb = st.tile([128, 1], F32)
            nc.vector.memset(zb[:], 0.0)
            ident = st.tile([128, 128], F32)
            make_identity(nc, ident[:])
            iot112 = st.tile([KP, 1], F32)
            nc.gpsimd.iota(iot112[:], pattern=[[0, 1]], base=1,
                           channel_multiplier=1,
                           allow_small_or_imprecise_dtypes=True)
            iot128 = st.tile([128, 1], F32)
            nc.gpsimd.iota(iot128[:], pattern=[[0, 1]], base=1,
                           channel_multiplier=1,
                           allow_small_or_imprecise_dtypes=True)
            ones640 = onef[:].to_broadcast([128, NB * N_OUT])

            # x resident [112, KT*2048] bf16 (28KB/partition)
            xsb = st.tile([KP, KT * BS], BF16)
            for kt in range(KT):
                nc.sync.dma_start(xsb[:, kt * BS:(kt + 1) * BS], xT[kt])

            # ================= s2 threshold (exact) =================
            s2ctx = tc.tile_pool(name="s2p", bufs=1)
            s2p = s2ctx.__enter__()
            s2sb = s2p.tile([128, NB * N_OUT], F32)
            nc.sync.dma_start(s2sb[:], s2r[:])
            w2raw = s2p.tile([128, NB * N_OUT], BF16)
            nc.sync.dma_start(w2raw[:], w2r[:])
            a2 = s2p.tile([128, NB * N_OUT], F32)
            nc.vector.tensor_scalar(a2[:].bitcast(U32), s2sb[:].bitcast(U32),
                                    0x7FFFFFFF, scalar2=None,
                                    op0=ALU.bitwise_and)
            scr2 = s2p.tile([128, NB * N_OUT], F32)
            rm2 = sm.tile([128, 1], F32, tag="rm2")
            nc.vector.tensor_reduce(rm2[:], a2[:], axis=AX.X, op=ALU.max)
            gmax2 = st.tile([128, 1], F32)
            nc.gpsimd.partition_all_reduce(gmax2[:], rm2[:], channels=128,
                                           reduce_op=ReduceOp.max)
            gridS1 = _mkgrid(nc, sm, iot128, zb, gmax2, 128, tag="s2a")
            c2a = sm.tile([128, 1], F32, tag="c2a")
            nc.vector.scalar_tensor_tensor(
                scr2[:], a2[:], gridS1[:, :1], ones640, op0=ALU.is_lt,
                op1=ALU.mult, accum_out=c2a[:])
            chat2 = sm.tile([128, 1], F32, tag="chat2")
            nc.vector.tensor_scalar(chat2[:], c2a[:], 128.0, scalar2=None,
                                    op0=ALU.mult)
            L2, U2 = _bracket(nc, sm, gridS1, chat2, float(J2 - M2A),
                              float(J2 + M2A), zb, gmax2, 128, tag="s2b")
            # refine: exact counts below L2/U2 + grid counts
            gridS2 = _mkgrid(nc, sm, iot128, L2, U2, 128, tag="s2c")
            cL2 = sm.tile([128, 1], F32, tag="cL2")
            nc.vector.scalar_tensor_tensor(
                scr2[:], a2[:], L2[:, :1], ones640, op0=ALU.is_lt,
                op1=ALU.mult, accum_out=cL2[:])
            cU2 = sm.tile([128, 1], F32, tag="cU2")
            nc.vector.scalar_tensor_tensor(
                scr2[:], a2[:], U2[:, :1], ones640, op0=ALU.is_lt,
                op1=ALU.mult, accum_out=cU2[:])
            cg2 = sm.tile([128, 1], F32, tag="cg2")
            nc.vector.scalar_tensor_tensor(
                scr2[:], a2[:], gridS2[:, :1], ones640, op0=ALU.is_lt,
                op1=ALU.mult, accum_out=cg2[:])
            CL2 = st.tile([128, 1], F32)
            nc.gpsimd.partition_all_reduce(CL2[:], cL2[:], channels=128,
                                           reduce_op=ReduceOp.add)
            CU2 = st.tile([128, 1], F32)
            nc.gpsimd.partition_all_reduce(CU2[:], cU2[:], channels=128,
                                           reduce_op=ReduceOp.add)
            d2 = sm.tile([128, 1], F32, tag="d2")
            nc.vector.tensor_tensor(d2[:], cg2[:], cL2[:], op=ALU.subtract)
            S2 = st.tile([128, 1], F32)
            nc.gpsimd.partition_all_reduce(S2[:], d2[:], channels=128,
                                           reduce_op=ReduceOp.add)
            # t_hat = midW + (J2 - (CL2 + S2)) * (U2-L2)/(CU2-CL2)
            wid2 = sm.tile([128, 1], F32, tag="wid2")
            nc.vector.tensor_tensor(wid2[:], U2[:], L2[:], op=ALU.subtract)
            den2 = sm.tile([128, 1], F32, tag="den2")
            nc.vector.tensor_tensor(den2[:], CU2[:], CL2[:], op=ALU.subtract)
            rho2i = sm.tile([128, 1], F32, tag="rho2i")
            nc.vector.reciprocal(rho2i[:], den2[:])
            nc.vector.tensor_tensor(rho2i[:], rho2i[:], wid2[:], op=ALU.mult)
            mid2 = sm.tile([128, 1], F32, tag="mid2")
            nc.vector.tensor_scalar(mid2[:], wid2[:], 129.0 / 256.0,
                                    scalar2=None, op0=ALU.mult)
            nc.vector.tensor_tensor(mid2[:], mid2[:], L2[:], op=ALU.add)
            rr2 = sm.tile([128, 1], F32, tag="rr2")
            nc.vector.tensor_tensor(rr2[:], CL2[:], S2[:], op=ALU.add)
            nc.vector.tensor_scalar(rr2[:], rr2[:], -1.0, scalar2=float(J2),
                                    op0=ALU.mult, op1=ALU.add)
            that2 = sm.tile([128, 1], F32, tag="that2")
            nc.vector.tensor_tensor(that2[:], rr2[:], rho2i[:], op=ALU.mult)
            nc.vector.tensor_tensor(that2[:], that2[:], mid2[:], op=ALU.add)
            mrg2 = sm.tile([128, 1], F32, tag="mrg2")
            nc.vector.tensor_scalar(mrg2[:], rho2i[:], M2B, scalar2=None,
                                    op0=ALU.mult)
            T2lo = st.tile([128, 1], F32)
            nc.vector.tensor_tensor(T2lo[:], that2[:], mrg2[:],
                                    op=ALU.subtract)
            T2hi = st.tile([128, 1], F32)
            nc.vector.tensor_tensor(T2hi[:], that2[:], mrg2[:], op=ALU.add)
            # exact count below T2lo
            cb2 = sm.tile([128, 1], F32, tag="cb2")
            nc.vector.scalar_tensor_tensor(
                scr2[:], a2[:], T2lo[:, :1], ones640, op0=ALU.is_lt,
                op1=ALU.mult, accum_out=cb2[:])
            CB2 = st.tile([128, 1], F32)
            nc.gpsimd.partition_all_reduce(CB2[:], cb2[:], channels=128,
                                           reduce_op=ReduceOp.add)
            # band extraction (lossless): z2 = (a2 < T2hi)*a2, iterated max8
            z2 = s2p.tile([128, NB * N_OUT], F32)
            nc.vector.scalar_tensor_tensor(z2[:], a2[:], T2hi[:, :1], a2[:],
                                           op0=ALU.is_lt, op1=ALU.mult)
            B2s = s2p.tile([128, MX2 * 8], F32)
            src = z2
            for i in range(MX2):
                mx = B2s[:, i * 8:(i + 1) * 8]
                nc.vector.max(out=mx, in_=src[:])
                if i < MX2 - 1:
                    nxt = s2p.tile([128, NB * N_OUT], F32,
                                   name=f"s2mr{i}", tag=f"s2mr{i % 2}")
                    nc.vector.match_replace(out=nxt[:], in_to_replace=mx,
                                            in_values=src[:], imm_value=-1.0)
                    src = nxt
            # gather band to partition 0, broadcast, refine rounds
            g2 = s2p.tile([1, 128 * MX2 * 8], F32)
            nc.sync.dma_start(g2[:], B2s[:])
            gb2 = s2p.tile([128, 128 * MX2 * 8], F32)
            nc.gpsimd.partition_broadcast(gb2[:], g2[:], channels=128)
            scrb2 = s2p.tile([128, 128 * MX2 * 8], BF16)
            onesg2 = onef[:].to_broadcast([128, 128 * MX2 * 8])
            grb2 = sm.tile([128, 1], F32, tag="grb2")
            nc.vector.scalar_tensor_tensor(
                scrb2[:], gb2[:], T2lo[:, :1], onesg2, op0=ALU.is_lt,
                op1=ALU.mult, accum_out=grb2[:])
            j2p = sm.tile([128, 1], F32, tag="j2p")
            nc.vector.tensor_scalar(j2p[:], CB2[:], -1.0, scalar2=float(J2),
                                    op0=ALU.mult, op1=ALU.add)
            nc.vector.tensor_tensor(j2p[:], j2p[:], grb2[:], op=ALU.add)
            v2 = _rounds_extract(nc, sm, gb2[:], scrb2[:], 128 * MX2 * 8, 128,
                                 iot128, onesg2, T2lo, T2hi, j2p, NR,
                                 tag="s2r")
            # masked w2 (a2 = |s2| already computed)
            pr2 = s2p.tile([128, NB * N_OUT], U32)
            nc.vector.tensor_scalar(pr2[:], a2[:], v2[:, :1], scalar2=None,
                                    op0=ALU.is_lt)
            w2m = st.tile([128, NB * N_OUT], BF16)
            nc.vector.select(w2m[:], pr2[:],
                             zbf16[:].to_broadcast([128, NB * N_OUT]),
                             w2raw[:])
            s2ctx.__exit__(None, None, None)

            # ================= s1 threshold (sharded) =================
            ones_ch = onef[:KP].to_broadcast([KP, 4096])
            ones_sh = onef[:KP].to_broadcast([KP, SH])
            with (
                tc.tile_pool(name="thr", bufs=1) as thp,
                tc.tile_pool(name="dramb", bufs=1, space="DRAM") as drb,
            ):
                with tc.tile_pool(name="pA", bufs=1) as pA:
                    # ---- pass A: replicated subset [112, 4096] ----
                    rawA = pA.tile([KP, 4096], F32)
                    nc.sync.dma_start(rawA[:], s1r[:, 0:4096])
                    aA = pA.tile([KP, 4096], F32)
                    nc.scalar.activation(aA[:], rawA[:], AF.Abs, bias=0.0,
                                         scale=1.0)
                    rmax = sm.tile([KP, 1], F32, tag="rmax")
                    nc.vector.tensor_reduce(rmax[:], aA[:], axis=AX.X,
                                            op=ALU.max)
                    gmax1 = st.tile([KP, 1], F32)
                    nc.gpsimd.partition_all_reduce(gmax1[:], rmax[:],
                                                   channels=KP,
                                                   reduce_op=ReduceOp.max)
                    gridA1 = _mkgrid(nc, sm, iot112, zb[:KP], gmax1, KP,
                                     tag="a1")
                    cA1 = sm.tile([KP, 1], F32, tag="cA1")
                    nc.vector.scalar_tensor_tensor(
                        rawA[:], aA[:], gridA1[:, :1], ones_ch, op0=ALU.is_lt,
                        op1=ALU.mult, accum_out=cA1[:])
                    chatA = sm.tile([KP, 1], F32, tag="chatA")
                    nc.vector.tensor_scalar(chatA[:], cA1[:], SUBF,
                                            scalar2=None, op0=ALU.mult)
                    LA, UA = _bracket(nc, sm, gridA1, chatA, float(J1 - MA1),
                                      float(J1 + MA1), zb[:KP], gmax1, KP,
                                      tag="aB")
                    # ---- pass A2: anchored S-sum interpolation on subset ----
                    gridA2 = _mkgrid(nc, sm, iot112, LA, UA, KP, tag="a2")
                    pkA = sm.tile([KP, 3], F32, tag="pkA")
                    nc.vector.scalar_tensor_tensor(
                        rawA[:], aA[:], LA[:, :1], ones_ch, op0=ALU.is_lt,
                        op1=ALU.mult, accum_out=pkA[:, 0:1])
                    nc.vector.scalar_tensor_tensor(
                        rawA[:], aA[:], UA[:, :1], ones_ch, op0=ALU.is_lt,
                        op1=ALU.mult, accum_out=pkA[:, 1:2])
                    nc.vector.scalar_tensor_tensor(
                        rawA[:], aA[:], gridA2[:, :1], ones_ch, op0=ALU.is_lt,
                        op1=ALU.mult, accum_out=pkA[:, 2:3])
                    tA = sm.tile([KP, 3], F32, tag="tA")
                    nc.gpsimd.partition_all_reduce(tA[:], pkA[:], channels=KP,
                                                   reduce_op=ReduceOp.add)
                    Tlo, Thi = _interp_band(nc, sm, st, tA[:, 0:1],
                                            tA[:, 1:2], tA[:, 2:3], LA, UA,
                                            KP, SUBF / 112.0, MA2, tag="A")

                # ---- pass B': shard-resident counts + AllReduce ----
                sh = thp.tile([KP, SH], F32)
                nc.sync.dma_start(sh[:], s1sh[:])
                ash = thp.tile([KP, SH], F32)
                nc.scalar.activation(ash[:], sh[:], AF.Abs, bias=0.0,
                                     scale=1.0)
                scrB = thp.tile([KP, SH], F32)
                gridB = _mkgrid(nc, sm, iot112, Tlo, Thi, KP, tag="b")
                pkB = thp.tile([KP, 3], F32)
                nc.vector.scalar_tensor_tensor(
                    scrB[:], ash[:], Tlo[:, :1], ones_sh, op0=ALU.is_lt,
                    op1=ALU.mult, accum_out=pkB[:, 0:1])
                nc.vector.scalar_tensor_tensor(
                    scrB[:], ash[:], Thi[:, :1], ones_sh, op0=ALU.is_lt,
                    op1=ALU.mult, accum_out=pkB[:, 1:2])
                nc.vector.scalar_tensor_tensor(
                    scrB[:], ash[:], gridB[:, :1], ones_sh, op0=ALU.is_lt,
                    op1=ALU.mult, accum_out=pkB[:, 2:3])
                bi3 = drb.tile([KP, 3], F32)
                bo3 = drb.tile([KP, 3], F32)
                nc.gpsimd.dma_start(bi3[:], pkB[:])
                nc.gpsimd.collective_compute(
                    "AllReduce", ALU.add,
                    replica_groups=[list(range(N_CORES))],
                    ins=[bi3[:].opt()], outs=[bo3[:].opt()])
                g3 = thp.tile([KP, 3], F32)
                nc.gpsimd.dma_start(g3[:], bo3[:])
                t3 = thp.tile([KP, 3], F32)
                nc.gpsimd.partition_all_reduce(t3[:], g3[:], channels=KP,
                                               reduce_op=ReduceOp.add)
                T3lo, T3hi = _interp_band(nc, sm, st, t3[:, 0:1], t3[:, 1:2],
                                          t3[:, 2:3], Tlo, Thi, KP, 1.0, MB1,
                                          tag="B")

                # ---- pass C': shard band extraction + AllGather ----
                pk2 = thp.tile([KP, 32], F32)
                nc.vector.memset(pk2[:], 0.0)
                nc.vector.scalar_tensor_tensor(
                    scrB[:], ash[:], T3lo[:, :1], ones_sh, op0=ALU.is_lt,
                    op1=ALU.mult, accum_out=pk2[:, 28:29])
                # z = (|a| < T3hi) * |a| written over the raw shard tile
                nc.vector.scalar_tensor_tensor(
                    sh[:], ash[:], T3hi[:, :1], ash[:], op0=ALU.is_lt,
                    op1=ALU.mult)
                zq = thp.tile([KP, SH // 16], F32)
                nc.vector.tensor_reduce(
                    zq[:], sh[:].rearrange("p (g k) -> p g k", k=16),
                    axis=AX.X, op=ALU.max)
                nc.vector.tensor_reduce(
                    pk2[:, 0:28], zq[:].rearrange("p (g k) -> p g k", k=16),
                    axis=AX.X, op=ALU.max)
                bi32 = drb.tile([KP, 32], F32)
                bo32 = drb.tile([N_CORES, KP, 32], F32)
                nc.gpsimd.dma_start(bi32[:], pk2[:])
                nc.gpsimd.collective_compute(
                    "AllGather", ALU.bypass,
                    replica_groups=[list(range(N_CORES))],
                    ins=[bi32[:].opt()], outs=[bo32[:].opt()])
                zu = thp.tile([KP, N_CORES * 28], F32)
                nc.gpsimd.dma_start(
                    zu[:].rearrange("p (c j) -> p c j", j=28),
                    bo32[:, :, 0:28].rearrange("c p j -> p c j"))
                c3c = thp.tile([KP, N_CORES], F32)
                nc.gpsimd.dma_start(
                    c3c[:].rearrange("p (c j) -> p c j", j=1),
                    bo32[:, :, 28:29].rearrange("c p j -> p c j"))
                c3s = sm.tile([KP, 1], F32, tag="c3s")
                nc.vector.tensor_reduce(c3s[:], c3c[:], axis=AX.X, op=ALU.add)
                C3 = st.tile([KP, 1], F32)
                nc.gpsimd.partition_all_reduce(C3[:], c3s[:], channels=KP,
                                               reduce_op=ReduceOp.add)

                # ---- rounds: compact union to top-24/row, broadcast ----
                B2u = thp.tile([KP, 24], F32)
                srcu = zu
                for i in range(3):
                    mxs = B2u[:, i * 8:(i + 1) * 8]
                    nc.vector.max(out=mxs, in_=srcu[:])
                    if i < 2:
                        nxtu = thp.tile([KP, N_CORES * 28], F32,
                                        name=f"s1mr{i}")
                        nc.vector.match_replace(out=nxtu[:], in_to_replace=mxs,
                                                in_values=srcu[:],
                                                imm_value=-1.0)
                        srcu = nxtu
                WB = KP * 24  # 2688
                gbr = thp.tile([KP, WB], F32)
                nc.sync.dma_start(gbr[0:1, :], B2u[:])
                nc.gpsimd.partition_broadcast(gbr[:], gbr[0:1, :],
                                              channels=KP)
                scrR = thp.tile([KP, WB], BF16)
                onesW = onef[:KP].to_broadcast([KP, WB])
                grb = sm.tile([KP, 1], F32, tag="grb")
                nc.vector.scalar_tensor_tensor(
                    scrR[:], gbr[:], T3lo[:, :1], onesW, op0=ALU.is_lt,
                    op1=ALU.mult, accum_out=grb[:])
                j1p = sm.tile([KP, 1], F32, tag="j1p")
                nc.vector.tensor_scalar(j1p[:], C3[:], -1.0,
                                        scalar2=float(J1), op0=ALU.mult,
                                        op1=ALU.add)
                nc.vector.tensor_tensor(j1p[:], j1p[:], grb[:], op=ALU.add)
                v1 = _rounds_extract(nc, sm, gbr[:], scrR[:], WB, KP,
                                     iot112, onesW, T3lo, T3hi, j1p, 2,
                                     tag="s1r")
                v1s = st.tile([KP, 1], F32)
                nc.vector.tensor_copy(v1s[:], v1[:])

            # ================= matmul pipeline =================
            lgps = [psl.tile([N_OUT, BBS], F32, tag=f"lg{bb}", name=f"lg{bb}")
                    for bb in range(NBB)]
            for nb in range(NB):
                w1b = mmp.tile([KP, KT * 128], BF16, tag="w1b")
                nc.sync.dma_start(w1b[:],
                                  w1r[:, nb * KT * 128:(nb + 1) * KT * 128])
                s1b = mmp.tile([KP, KT * 128], F32, tag="s1b")
                nc.sync.dma_start(s1b[:],
                                  s1r[:, nb * KT * 128:(nb + 1) * KT * 128])
                nc.vector.tensor_scalar(s1b[:].bitcast(U32),
                                        s1b[:].bitcast(U32), 0x7FFFFFFF,
                                        scalar2=None, op0=ALU.bitwise_and)
                nc.vector.tensor_scalar(s1b[:].bitcast(U32), s1b[:],
                                        v1s[:, :1], scalar2=None,
                                        op0=ALU.is_lt)
                w1m = mmp.tile([KP, KT * 128], BF16, tag="w1m")
                nc.vector.select(w1m[:], s1b[:].bitcast(U32),
                                 zbf16[:KP].to_broadcast([KP, KT * 128]),
                                 w1b[:])
                hts = []
                for bb in range(NBB):
                    ph = psh.tile([128, BBS], F32, tag="ph")
                    for kt in range(KT):
                        nc.tensor.matmul(
                            ph[:], w1m[:, kt * 128:(kt + 1) * 128],
                            xsb[:, kt * BS + bb * BBS:
                                kt * BS + (bb + 1) * BBS],
                            start=(kt == 0), stop=(kt == KT - 1))
                    ht = hbp.tile([128, BBS], BF16, tag="ht")
                    nc.scalar.activation(ht[:], ph[:], AF.Relu, bias=0.0,
                                         scale=1.0)
                    hts.append(ht)
                w2s = w2m[:, nb * N_OUT:(nb + 1) * N_OUT]
                for bb in range(NBB):
                    nc.tensor.matmul(lgps[bb][:], w2s, hts[bb][:],
                                     start=(nb == 0), stop=(nb == NB - 1),
                                     skip_group_check=True)

            # ================= epilogue: log_softmax =================
            lga = epi.tile([128, 16 * N_OUT], F32, tag="lga")
            for bb in range(NBB):
                lg = epi.tile([N_OUT, BBS], F32, tag="lgc")
                nc.vector.tensor_copy(lg[:], lgps[bb][:])
                for c in range(BBS // 128):
                    g = bb * 4 + c
                    pt = psh.tile([128, BBS], F32, tag="ph")
                    nc.tensor.transpose(pt[:, :N_OUT],
                                        lg[:, c * 128:(c + 1) * 128],
                                        ident[:N_OUT, :N_OUT])
                    nc.vector.tensor_copy(lga[:, g * N_OUT:(g + 1) * N_OUT],
                                          pt[:, :N_OUT])
            lga3 = lga[:].rearrange("p (g k) -> p g k", k=N_OUT)
            mx = epi.tile([128, 16], F32, tag="mx")
            nc.vector.tensor_reduce(mx[:], lga3, axis=AX.X, op=ALU.max)
            mxb = mx[:].unsqueeze(2).to_broadcast([128, 16, N_OUT])
            nc.vector.tensor_tensor(lga3, lga3, mxb, op=ALU.subtract)
            ex = epi.tile([128, 16 * N_OUT], F32, tag="ex")
            nc.scalar.activation(ex[:], lga[:], AF.Exp, bias=0.0, scale=1.0)
            se = epi.tile([128, 16], F32, tag="se")
            nc.vector.tensor_reduce(se[:],
                                    ex[:].rearrange("p (g k) -> p g k",
                                                    k=N_OUT),
                                    axis=AX.X, op=ALU.add)
            ls = epi.tile([128, 16], F32, tag="ls")
            nc.scalar.activation(ls[:], se[:], AF.Ln, bias=zb[:, :1],
                                 scale=1.0)
            lsb = ls[:].unsqueeze(2).to_broadcast([128, 16, N_OUT])
            nc.vector.tensor_tensor(lga3, lga3, lsb, op=ALU.subtract)
            for g in range(16):
                nc.sync.dma_start(out[g * 128:(g + 1) * 128, :],
                                  lga[:, g * N_OUT:(g + 1) * N_OUT])
    nc.compile()
    return nc


def _prep_inputs(x, w1, s1, w2, s2):
    bf = ml_dtypes.bfloat16
    w1r = np.ascontiguousarray(
        w1.reshape(NB, 128, KT, KP).transpose(3, 0, 2, 1).reshape(KP, WCOL)
    ).astype(bf)
    s1r = np.ascontiguousarray(
        s1.reshape(NB, 128, KT, KP).transpose(3, 0, 2, 1).reshape(KP, WCOL)
    ).astype(np.float32)
    w2r = np.ascontiguousarray(
        w2.T.reshape(NB, 128, N_OUT).transpose(1, 0, 2).reshape(128,
                                                                NB * N_OUT)
    ).astype(bf)
    s2r = np.ascontiguousarray(
        s2.T.reshape(NB, 128, N_OUT).transpose(1, 0, 2).reshape(128,
                                                                NB * N_OUT)
    ).astype(np.float32)
    in_maps = []
    for cid in range(N_CORES):
        xc = np.ascontiguousarray(
            x[cid * BS:(cid + 1) * BS].T).reshape(KT, KP, BS).astype(bf)
        shc = np.ascontiguousarray(s1r[:, cid * SH:(cid + 1) * SH])
        in_maps.append({"xT": xc, "w1r": w1r, "s1r": s1r, "s1sh": shc,
                        "w2r": w2r, "s2r": s2r})
    return in_maps


def kernel(x, w1, s1, w2, s2):
    x = np.asarray(x); w1 = np.asarray(w1); s1 = np.asarray(s1)
    w2 = np.asarray(w2); s2 = np.asarray(s2)
    if "nc" not in _cache:
        _cache["nc"] = build_program()
    nc = _cache["nc"]
    in_maps = _prep_inputs(x, w1, s1, w2, s2)
    res = run_bass_kernel_spmd(nc, in_maps, list(range(N_CORES)))
    return np.concatenate([res.results[c]["out"] for c in range(N_CORES)],
                          axis=0)


if __name__ == "__main__":
    sys.path.insert(0, "/root/problem")
    from reference import setup_inputs
    inputs = {k: np.asarray(v) for k, v in setup_inputs().items()}
    got = kernel(**inputs)
    print("out", got.shape, got.dtype)
    print(got[:2])


# revision 17
# speedup vs baseline: 2.0792x; 1.0129x over previous
"""Trainium2 Bass kernel for nn_Net_39041252721137 (supermask MLP with global
top-50% |score| masking).

Data-parallel on batch across 8 cores; replicated scores/weights. Global
top-k thresholds computed ON DEVICE per core with a count-based scheme
(all exact counts, no per-element sort):

  s1 (6.4M elements):
    A  subset (1/7) stratified estimate -> bracket [Tlo, Thi] (~±50k ranks)
    B  full stream: exact counts at Tlo/Thi + per-partition grid counts
       -> interpolated t_hat (sigma ~1.4e2 ranks) -> band3 [T3lo, T3hi]
    C  full stream: exact count below T3lo + suppress >=T3hi + 3-level
       max-pool compaction of the ~1.1k-element band into [112,112]
    R  gather band to one partition, broadcast to all, 3 stratified
       rounds of exact counting -> rank-J1 value v1 (pool collisions can
       shift the rank by a few tens; output effect ~1e-4)
  s2 (82k elements): same idea, but the band extraction is lossless
    (iterated max8/match_replace on the small resident tile) -> exact v2.

Then masked bf16 matmuls: h = relu(x @ (w1*m1).T), logits = h @ (w2*m2).T,
log_softmax fused in one pass over 64 neuron blocks.
"""
import sys

import numpy as np
import ml_dtypes

sys.path.insert(0, "/root/.axon_site")

import concourse.bass as bass
import concourse.bacc as bacc
import concourse.mybir as mybir
import concourse.tile as tile
from concourse.bass_isa import ReduceOp
from concourse.bass_utils import run_bass_kernel_spmd
from concourse.masks import make_identity

F32 = mybir.dt.float32
BF16 = mybir.dt.bfloat16
U32 = mybir.dt.uint32
AF = mybir.ActivationFunctionType
ALU = mybir.AluOpType
AX = mybir.AxisListType

N_CORES = 8
B, D_IN, N2, N_OUT = 16384, 784, 8192, 10
BS = B // N_CORES            # 2048 batch rows per core
KT, KP = 7, 112              # d_in tiled as 7 x 112 partitions
NB = N2 // 128               # 64 neuron blocks
WCOL = NB * KT * 128         # 57344 = per-partition columns of w1r/s1r
CHW = 4096                   # threshold streaming chunk width
NCH = WCOL // CHW            # 14 chunks
SH = WCOL // N_CORES         # 7168 shard columns per core
N1 = N2 * D_IN               # 6422528
SUBF = float(N1 // CHW)      # subset per-point extrapolation factor (1568)
J1 = N1 // 2
NS2 = N_OUT * N2             # 81920
J2 = NS2 // 2
BBS = 512
NBB = BS // BBS              # 4

MA1 = 250000.0               # s1 pass-A1 bracket margin (ranks)
MA2 = 24000.0                # s1 pass-A2 band half-width (ranks, ~5 sigma)
MB1 = 520.0                  # s1 band3 half-width (ranks)
M2A = 8000.0                 # s2 coarse bracket margin (ranks)
M2B = 350.0                  # s2 band half-width (ranks)
NR = 3                       # stratified refinement rounds (each /P width)
MX2 = 3                      # s2 max8 iterations (capacity 24/row)

_cache = {}


def _bracket(nc, pool, grid, est, jlo, jhi, Lfb, Ufb, P, tag):
    """[L, U] = (max grid pt with est<jlo, min grid pt with est>jhi),
    falling back to Lfb/Ufb. All tiles [P,1] f32; est compared to imms."""
    selL = pool.tile([P, 1], U32, tag=f"{tag}sl")
    nc.vector.tensor_scalar(selL[:], est[:], jlo, scalar2=None, op0=ALU.is_lt)
    candL = pool.tile([P, 1], F32, tag=f"{tag}cl")
    nc.vector.select(candL[:], selL[:], grid[:], Lfb[:])
    L = pool.tile([P, 1], F32, tag=f"{tag}L")
    nc.gpsimd.partition_all_reduce(L[:], candL[:], channels=P,
                                   reduce_op=ReduceOp.max)
    selU = pool.tile([P, 1], U32, tag=f"{tag}su")
    nc.vector.tensor_scalar(selU[:], est[:], jhi, scalar2=None, op0=ALU.is_gt)
    candU = pool.tile([P, 1], F32, tag=f"{tag}cu")
    nc.vector.select(candU[:], selU[:], grid[:], Ufb[:])
    nc.vector.tensor_scalar(candU[:], candU[:], -1.0, scalar2=None,
                            op0=ALU.mult)
    U = pool.tile([P, 1], F32, tag=f"{tag}U")
    nc.gpsimd.partition_all_reduce(U[:], candU[:], channels=P,
                                   reduce_op=ReduceOp.max)
    nc.vector.tensor_scalar(U[:], U[:], -1.0, scalar2=None, op0=ALU.mult)
    return L, U


def _mkgrid(nc, pool, iot, L, U, P, tag):
    """grid_p = L + p*(U-L)/P for p=1..P (t_P == U)."""
    g = pool.tile([P, 1], F32, tag=f"{tag}g")
    nc.vector.tensor_tensor(g[:], U[:], L[:], op=ALU.subtract)
    nc.vector.tensor_scalar(g[:], g[:], 1.0 / P, scalar2=None, op0=ALU.mult)
    nc.vector.tensor_tensor(g[:], iot[:], g[:], op=ALU.mult)
    nc.vector.tensor_tensor(g[:], g[:], L[:], op=ALU.add)
    return g


def _interp_band(nc, pool, st, cloAP, chiAP, cgAP, L, U, P, scale, margin,
                 jtarget, tag):
    """Anchored S-sum interpolation: counts (already summed over partitions
    and cores) at L, U, and the P-point grid spanning [L, U]; returns band
    [lo, hi] = t_hat -+ margin ranks around the rank-J1 interpolant.
    scale converts counts to full-data ranks."""
    wid = pool.tile([P, 1], F32, tag=f"{tag}w")
    nc.vector.tensor_tensor(wid[:], U[:], L[:], op=ALU.subtract)
    den = pool.tile([P, 1], F32, tag=f"{tag}d")
    nc.vector.tensor_tensor(den[:], chiAP, cloAP, op=ALU.subtract)
    nc.vector.tensor_scalar(den[:], den[:], scale, scalar2=None, op0=ALU.mult)
    rhoi = pool.tile([P, 1], F32, tag=f"{tag}ri")
    nc.vector.reciprocal(rhoi[:], den[:])
    nc.vector.tensor_tensor(rhoi[:], rhoi[:], wid[:], op=ALU.mult)
    mid = pool.tile([P, 1], F32, tag=f"{tag}m")
    nc.vector.tensor_scalar(mid[:], wid[:], (P + 1.0) / (2.0 * P),
                            scalar2=None, op0=ALU.mult)
    nc.vector.tensor_tensor(mid[:], mid[:], L[:], op=ALU.add)
    rr = pool.tile([P, 1], F32, tag=f"{tag}rr")
    nc.vector.tensor_scalar(rr[:], cgAP, -scale, scalar2=float(jtarget),
                            op0=ALU.mult, op1=ALU.add)
    that = pool.tile([P, 1], F32, tag=f"{tag}t")
    nc.vector.tensor_tensor(that[:], rr[:], rhoi[:], op=ALU.mult)
    nc.vector.tensor_tensor(that[:], that[:], mid[:], op=ALU.add)
    mrg = pool.tile([P, 1], F32, tag=f"{tag}mg")
    nc.vector.tensor_scalar(mrg[:], rhoi[:], margin, scalar2=None,
                            op0=ALU.mult)
    lo = st.tile([P, 1], F32, name=f"{tag}lo")
    nc.vector.tensor_tensor(lo[:], that[:], mrg[:], op=ALU.subtract)
    hi = st.tile([P, 1], F32, name=f"{tag}hi")
    nc.vector.tensor_tensor(hi[:], that[:], mrg[:], op=ALU.add)
    return lo, hi


def _rounds_extract(nc, pool, gb_ap, scr_ap, W, P, iot, onesW, L0, U0, jp,
                    n_rounds, tag):
    """n_rounds stratified rounds of exact counting on broadcast data, then
    extract the unique representable value in the final [L, U)."""
    L, U = L0, U0
    for r in range(n_rounds):
        grid = _mkgrid(nc, pool, iot, L, U, P, tag=f"{tag}r")
        cR = pool.tile([P, 1], F32, tag=f"{tag}c")
        nc.vector.scalar_tensor_tensor(
            scr_ap, gb_ap, grid[:, :1], onesW, op0=ALU.is_lt, op1=ALU.mult,
            accum_out=cR[:])
        selL = pool.tile([P, 1], U32, tag=f"{tag}sl")
        nc.vector.tensor_tensor(selL[:], cR[:], jp[:], op=ALU.is_le)
        candL = pool.tile([P, 1], F32, tag=f"{tag}cl")
        nc.vector.select(candL[:], selL[:], grid[:], L[:])
        Ln = pool.tile([P, 1], F32, tag=f"{tag}L")
        nc.gpsimd.partition_all_reduce(Ln[:], candL[:], channels=P,
                                       reduce_op=ReduceOp.max)
        selU = pool.tile([P, 1], U32, tag=f"{tag}su")
        nc.vector.tensor_tensor(selU[:], cR[:], jp[:], op=ALU.is_gt)
        candU = pool.tile([P, 1], F32, tag=f"{tag}cu")
        nc.vector.select(candU[:], selU[:], grid[:], U[:])
        nc.vector.tensor_scalar(candU[:], candU[:], -1.0, scalar2=None,
                                op0=ALU.mult)
        Un = pool.tile([P, 1], F32, tag=f"{tag}U")
        nc.gpsimd.partition_all_reduce(Un[:], candU[:], channels=P,
                                       reduce_op=ReduceOp.max)
        nc.vector.tensor_scalar(Un[:], Un[:], -1.0, scalar2=None,
                                op0=ALU.mult)
        L, U = Ln, Un
    # v = max over values < U (the single representable value in [L, U))
    nc.vector.scalar_tensor_tensor(gb_ap, gb_ap, U[:, :1], gb_ap,
                                   op0=ALU.is_lt, op1=ALU.mult)
    v = pool.tile([P, 1], F32, tag=f"{tag}v")
    nc.vector.tensor_reduce(v[:], gb_ap, axis=AX.X, op=ALU.max)
    return v


def build_program():
    nc = bacc.Bacc("TRN2", target_bir_lowering=False, debug=False,
                   num_devices=N_CORES)

    xT = nc.declare_dram_parameter("xT", [KT, KP, BS], BF16, isOutput=False)
    w1r = nc.declare_dram_parameter("w1r", [KP, WCOL], BF16, isOutput=False)
    s1r = nc.declare_dram_parameter("s1r", [KP, WCOL], F32, isOutput=False)
    s1sh = nc.declare_dram_parameter("s1sh", [KP, SH], F32, isOutput=False)
    w2r = nc.declare_dram_parameter("w2r", [128, NB * N_OUT], BF16,
                                    isOutput=False)
    s2r = nc.declare_dram_parameter("s2r", [128, NB * N_OUT], F32,
                                    isOutput=False)
    out = nc.declare_dram_parameter("out", [BS, N_OUT], F32, isOutput=True)

    with tile.TileContext(nc) as tc:
        with (
            tc.tile_pool(name="state", bufs=1) as st,
            tc.tile_pool(name="small", bufs=2) as sm,
            tc.tile_pool(name="s2p", bufs=1) as s2p,
            tc.tile_pool(name="thr", bufs=1) as thp,
            tc.tile_pool(name="dramb", bufs=1, space="DRAM") as drb,
            tc.tile_pool(name="mm", bufs=3) as mmp,
            tc.tile_pool(name="hbuf", bufs=8) as hbp,
            tc.tile_pool(name="psum_h", bufs=4, space="PSUM") as psh,
            tc.tile_pool(name="psum_l", bufs=1, space="PSUM") as psl,
            tc.tile_pool(name="epi", bufs=2) as epi,
        ):
            # ---- shared constants ----
            onef = st.tile([128, 1], F32)
            nc.vector.memset(onef[:], 1.0)
            zbf16 = st.tile([128, 1], BF16)
            nc.vector.memset(zbf16[:], 0.0)
            zb = st.tile([128, 1], F32)
            nc.vector.memset(zb[:], 0.0)
            ident = st.tile([128, 128], F32)
            make_identity(nc, ident[:])
            iot112 = st.tile([KP, 1], F32)
            nc.gpsimd.iota(iot112[:], pattern=[[0, 1]], base=1,
                           channel_multiplier=1,
                           allow_small_or_imprecise_dtypes=True)
            iot128 = st.tile([128, 1], F32)
            nc.gpsimd.iota(iot128[:], pattern=[[0, 1]], base=1,
                           channel_multiplier=1,
                           allow_small_or_imprecise_dtypes=True)
            ones640 = onef[:].to_broadcast([128, NB * N_OUT])
            ones_ch = onef[:KP].to_broadcast([KP, 4096])
            ones_sh = onef[:KP].to_broadcast([KP, SH])

            # x resident [112, KT*2048] bf16 (28KB/partition)
            xsb = st.tile([KP, KT * BS], BF16)
            for kt in range(KT):
                nc.sync.dma_start(xsb[:, kt * BS:(kt + 1) * BS], xT[kt])

            # shard tile: pass A scratch first, then the B'/C' shard
            sh = thp.tile([KP, SH], F32)

            # ====== s2 stage 1: load + coarse bracket ======
            s2sb = s2p.tile([128, NB * N_OUT], F32)
            nc.sync.dma_start(s2sb[:], s2r[:])
            w2raw = s2p.tile([128, NB * N_OUT], BF16)
            nc.sync.dma_start(w2raw[:], w2r[:])
            a2 = s2p.tile([128, NB * N_OUT], F32)
            nc.vector.tensor_scalar(a2[:].bitcast(U32), s2sb[:].bitcast(U32),
                                    0x7FFFFFFF, scalar2=None,
                                    op0=ALU.bitwise_and)
            scr2 = s2p.tile([128, NB * N_OUT], BF16)
            rm2 = sm.tile([128, 1], F32, tag="rm2")
            nc.vector.tensor_reduce(rm2[:], a2[:], axis=AX.X, op=ALU.max)
            gmax2 = st.tile([128, 1], F32)
            nc.gpsimd.partition_all_reduce(gmax2[:], rm2[:], channels=128,
                                           reduce_op=ReduceOp.max)
            gridS1 = _mkgrid(nc, sm, iot128, zb, gmax2, 128, tag="s2a")
            c2a = sm.tile([128, 1], F32, tag="c2a")
            nc.vector.scalar_tensor_tensor(
                scr2[:], a2[:], gridS1[:, :1], ones640, op0=ALU.is_lt,
                op1=ALU.mult, accum_out=c2a[:])
            chat2 = sm.tile([128, 1], F32, tag="chat2")
            nc.vector.tensor_scalar(chat2[:], c2a[:], 128.0, scalar2=None,
                                    op0=ALU.mult)
            L2, U2 = _bracket(nc, sm, gridS1, chat2, float(J2 - M2A),
                              float(J2 + M2A), zb, gmax2, 128, tag="s2b")

            # ====== s1 pass A: replicated subset in sh[:, :4096] ======
            nc.sync.dma_start(sh[:, 0:4096], s1r[:, 0:4096])
            with tc.tile_pool(name="pA", bufs=1) as pA:
                aA = pA.tile([KP, 4096], F32)
                nc.scalar.activation(aA[:], sh[:, 0:4096], AF.Abs, bias=0.0,
                                     scale=1.0)
                rmax = sm.tile([KP, 1], F32, tag="rmax")
                nc.vector.tensor_reduce(rmax[:], aA[:], axis=AX.X, op=ALU.max)
                gmax1 = st.tile([KP, 1], F32)
                nc.gpsimd.partition_all_reduce(gmax1[:], rmax[:], channels=KP,
                                               reduce_op=ReduceOp.max)
                gridA1 = _mkgrid(nc, sm, iot112, zb[:KP], gmax1, KP, tag="a1")
                cA1 = sm.tile([KP, 1], F32, tag="cA1")
                nc.vector.scalar_tensor_tensor(
                    sh[:, 0:4096], aA[:], gridA1[:, :1], ones_ch,
                    op0=ALU.is_lt, op1=ALU.mult, accum_out=cA1[:])
                chatA = sm.tile([KP, 1], F32, tag="chatA")
                nc.vector.tensor_scalar(chatA[:], cA1[:], SUBF, scalar2=None,
                                        op0=ALU.mult)
                LA, UA = _bracket(nc, sm, gridA1, chatA, float(J1 - MA1),
                                  float(J1 + MA1), zb[:KP], gmax1, KP,
                                  tag="aB")
                gridA2 = _mkgrid(nc, sm, iot112, LA, UA, KP, tag="a2")
                pkA = sm.tile([KP, 3], F32, tag="pkA")
                nc.vector.scalar_tensor_tensor(
                    sh[:, 0:4096], aA[:], LA[:, :1], ones_ch, op0=ALU.is_lt,
                    op1=ALU.mult, accum_out=pkA[:, 0:1])
                nc.vector.scalar_tensor_tensor(
                    sh[:, 0:4096], aA[:], UA[:, :1], ones_ch, op0=ALU.is_lt,
                    op1=ALU.mult, accum_out=pkA[:, 1:2])
                nc.vector.scalar_tensor_tensor(
                    sh[:, 0:4096], aA[:], gridA2[:, :1], ones_ch,
                    op0=ALU.is_lt, op1=ALU.mult, accum_out=pkA[:, 2:3])
                tA = sm.tile([KP, 3], F32, tag="tA")
                nc.gpsimd.partition_all_reduce(tA[:], pkA[:], channels=KP,
                                               reduce_op=ReduceOp.add)
                Tlo, Thi = _interp_band(nc, sm, st, tA[:, 0:1], tA[:, 1:2],
                                        tA[:, 2:3], LA, UA, KP, SUBF / 112.0,
                                        MA2, J1, tag="A")

            # ====== pass B': shard-resident counts + AllReduce launch ======
            nc.sync.dma_start(sh[:], s1sh[:])
            ash = thp.tile([KP, SH], F32)
            nc.scalar.activation(ash[:], sh[:], AF.Abs, bias=0.0, scale=1.0)
            gridB = _mkgrid(nc, sm, iot112, Tlo, Thi, KP, tag="b")
            pkB = thp.tile([KP, 3], F32)
            nc.vector.scalar_tensor_tensor(
                sh[:], ash[:], Tlo[:, :1], ones_sh, op0=ALU.is_lt,
                op1=ALU.mult, accum_out=pkB[:, 0:1])
            nc.vector.scalar_tensor_tensor(
                sh[:], ash[:], Thi[:, :1], ones_sh, op0=ALU.is_lt,
                op1=ALU.mult, accum_out=pkB[:, 1:2])
            nc.vector.scalar_tensor_tensor(
                sh[:], ash[:], gridB[:, :1], ones_sh, op0=ALU.is_lt,
                op1=ALU.mult, accum_out=pkB[:, 2:3])
            bi3 = drb.tile([KP, 3], F32)
            bo3 = drb.tile([KP, 3], F32)
            nc.gpsimd.dma_start(bi3[:], pkB[:])
            nc.gpsimd.collective_compute(
                "AllReduce", ALU.add,
                replica_groups=[list(range(N_CORES))],
                ins=[bi3[:].opt()], outs=[bo3[:].opt()])

            # ====== s2 stage 2 (hides under the AllReduce) ======
            gridS2 = _mkgrid(nc, sm, iot128, L2, U2, 128, tag="s2c")
            pk2s = sm.tile([128, 3], F32, tag="pk2s")
            nc.vector.scalar_tensor_tensor(
                scr2[:], a2[:], L2[:, :1], ones640, op0=ALU.is_lt,
                op1=ALU.mult, accum_out=pk2s[:, 0:1])
            nc.vector.scalar_tensor_tensor(
                scr2[:], a2[:], U2[:, :1], ones640, op0=ALU.is_lt,
                op1=ALU.mult, accum_out=pk2s[:, 1:2])
            nc.vector.scalar_tensor_tensor(
                scr2[:], a2[:], gridS2[:, :1], ones640, op0=ALU.is_lt,
                op1=ALU.mult, accum_out=pk2s[:, 2:3])
            tS = sm.tile([128, 3], F32, tag="tS")
            nc.gpsimd.partition_all_reduce(tS[:], pk2s[:], channels=128,
                                           reduce_op=ReduceOp.add)
            T2lo, T2hi = _interp_band(nc, sm, st, tS[:, 0:1], tS[:, 1:2],
                                      tS[:, 2:3], L2, U2, 128, 1.0, M2B, J2,
                                      tag="S")
            cb2 = sm.tile([128, 1], F32, tag="cb2")
            nc.vector.scalar_tensor_tensor(
                scr2[:], a2[:], T2lo[:, :1], ones640, op0=ALU.is_lt,
                op1=ALU.mult, accum_out=cb2[:])
            CB2 = st.tile([128, 1], F32)
            nc.gpsimd.partition_all_reduce(CB2[:], cb2[:], channels=128,
                                           reduce_op=ReduceOp.add)
            z2 = s2p.tile([128, NB * N_OUT], F32)
            nc.vector.scalar_tensor_tensor(z2[:], a2[:], T2hi[:, :1], a2[:],
                                           op0=ALU.is_lt, op1=ALU.mult)
            B2s = s2p.tile([128, MX2 * 8], F32)
            mr0 = s2p.tile([128, NB * N_OUT], F32)
            srcs = [z2, mr0, z2]
            for i in range(MX2):
                mx = B2s[:, i * 8:(i + 1) * 8]
                nc.vector.max(out=mx, in_=srcs[i][:])
                if i < MX2 - 1:
                    nc.vector.match_replace(out=srcs[i + 1][:],
                                            in_to_replace=mx,
                                            in_values=srcs[i][:],
                                            imm_value=-1.0)
            W2B = 128 * MX2 * 8
            gb2 = s2p.tile([128, W2B], F32)
            nc.sync.dma_start(gb2[0:1, :], B2s[:])
            nc.gpsimd.partition_broadcast(gb2[:], gb2[0:1, :], channels=128)
            scrb2 = s2p.tile([128, W2B], BF16)
            onesg2 = onef[:].to_broadcast([128, W2B])
            grb2 = sm.tile([128, 1], F32, tag="grb2")
            nc.vector.scalar_tensor_tensor(
                scrb2[:], gb2[:], T2lo[:, :1], onesg2, op0=ALU.is_lt,
                op1=ALU.mult, accum_out=grb2[:])
            j2p = sm.tile([128, 1], F32, tag="j2p")
            nc.vector.tensor_scalar(j2p[:], CB2[:], -1.0, scalar2=float(J2),
                                    op0=ALU.mult, op1=ALU.add)
            nc.vector.tensor_tensor(j2p[:], j2p[:], grb2[:], op=ALU.add)
            v2 = _rounds_extract(nc, sm, gb2[:], scrb2[:], W2B, 128,
                                 iot128, onesg2, T2lo, T2hi, j2p, NR,
                                 tag="s2r")
            pr2 = s2p.tile([128, NB * N_OUT], U32)
            nc.vector.tensor_scalar(pr2[:], a2[:], v2[:, :1], scalar2=None,
                                    op0=ALU.is_lt)
            w2m = st.tile([128, NB * N_OUT], BF16)
            nc.vector.select(w2m[:], pr2[:],
                             zbf16[:].to_broadcast([128, NB * N_OUT]),
                             w2raw[:])

            # ====== pass B' readback + interpolation ======
            g3 = thp.tile([KP, 3], F32)
            nc.gpsimd.dma_start(g3[:], bo3[:])
            t3 = thp.tile([KP, 3], F32)
            nc.gpsimd.partition_all_reduce(t3[:], g3[:], channels=KP,
                                           reduce_op=ReduceOp.add)
            T3lo, T3hi = _interp_band(nc, sm, st, t3[:, 0:1], t3[:, 1:2],
                                      t3[:, 2:3], Tlo, Thi, KP, 1.0, MB1, J1,
                                      tag="B")

            # ====== pass C': shard band extraction + AllGather ======
            pk2 = thp.tile([KP, 32], F32)
            nc.vector.memset(pk2[:], 0.0)
            nc.vector.scalar_tensor_tensor(
                sh[:], ash[:], T3lo[:, :1], ones_sh, op0=ALU.is_lt,
                op1=ALU.mult, accum_out=pk2[:, 28:29])
            # z = (|a| < T3hi) * |a| written over the raw shard tile
            nc.vector.scalar_tensor_tensor(
                sh[:], ash[:], T3hi[:, :1], ash[:], op0=ALU.is_lt,
                op1=ALU.mult)
            zq = thp.tile([KP, SH // 16], F32)
            nc.vector.tensor_reduce(
                zq[:], sh[:].rearrange("p (g k) -> p g k", k=16),
                axis=AX.X, op=ALU.max)
            nc.vector.tensor_reduce(
                pk2[:, 0:28], zq[:].rearrange("p (g k) -> p g k", k=16),
                axis=AX.X, op=ALU.max)
            bi32 = drb.tile([KP, 32], F32)
            bo32 = drb.tile([N_CORES, KP, 32], F32)
            nc.gpsimd.dma_start(bi32[:], pk2[:])
            nc.gpsimd.collective_compute(
                "AllGather", ALU.bypass,
                replica_groups=[list(range(N_CORES))],
                ins=[bi32[:].opt()], outs=[bo32[:].opt()])
            zu = thp.tile([KP, N_CORES * 28], F32)
            nc.gpsimd.dma_start(
                zu[:].rearrange("p (c j) -> p c j", j=28),
                bo32[:, :, 0:28].rearrange("c p j -> p c j"))
            c3c = thp.tile([KP, N_CORES], F32)
            nc.gpsimd.dma_start(
                c3c[:].rearrange("p (c j) -> p c j", j=1),
                bo32[:, :, 28:29].rearrange("c p j -> p c j"))
            c3s = sm.tile([KP, 1], F32, tag="c3s")
            nc.vector.tensor_reduce(c3s[:], c3c[:], axis=AX.X, op=ALU.add)
            C3 = st.tile([KP, 1], F32)
            nc.gpsimd.partition_all_reduce(C3[:], c3s[:], channels=KP,
                                           reduce_op=ReduceOp.add)

            # ====== s1 rounds: compact union to top-24/row, broadcast ======
            B2u = thp.tile([KP, 24], F32)
            mru = thp.tile([KP, N_CORES * 28], F32)
            srcu = [zu, mru, zu]
            for i in range(3):
                mxs = B2u[:, i * 8:(i + 1) * 8]
                nc.vector.max(out=mxs, in_=srcu[i][:])
                if i < 2:
                    nc.vector.match_replace(out=srcu[i + 1][:],
                                            in_to_replace=mxs,
                                            in_values=srcu[i][:],
                                            imm_value=-1.0)
            WB = KP * 24  # 2688
            gbr = thp.tile([KP, WB], F32)
            nc.sync.dma_start(gbr[0:1, :], B2u[:])
            nc.gpsimd.partition_broadcast(gbr[:], gbr[0:1, :], channels=KP)
            scrR = thp.tile([KP, WB], BF16)
            onesW = onef[:KP].to_broadcast([KP, WB])
            grb = sm.tile([KP, 1], F32, tag="grb")
            nc.vector.scalar_tensor_tensor(
                scrR[:], gbr[:], T3lo[:, :1], onesW, op0=ALU.is_lt,
                op1=ALU.mult, accum_out=grb[:])
            j1p = sm.tile([KP, 1], F32, tag="j1p")
            nc.vector.tensor_scalar(j1p[:], C3[:], -1.0, scalar2=float(J1),
                                    op0=ALU.mult, op1=ALU.add)
            nc.vector.tensor_tensor(j1p[:], j1p[:], grb[:], op=ALU.add)
            v1 = _rounds_extract(nc, sm, gbr[:], scrR[:], WB, KP, iot112,
                                 onesW, T3lo, T3hi, j1p, 2, tag="s1r")
            v1s = st.tile([KP, 1], F32)
            nc.vector.tensor_copy(v1s[:], v1[:])

            # ================= matmul pipeline =================
            lgps = [psl.tile([N_OUT, BBS], F32, tag=f"lg{bb}", name=f"lg{bb}")
                    for bb in range(NBB)]
            for nb in range(NB):
                w1b = mmp.tile([KP, KT * 128], BF16, tag="w1b")
                nc.sync.dma_start(w1b[:],
                                  w1r[:, nb * KT * 128:(nb + 1) * KT * 128])
                s1b = mmp.tile([KP, KT * 128], F32, tag="s1b")
                nc.sync.dma_start(s1b[:],
                                  s1r[:, nb * KT * 128:(nb + 1) * KT * 128])
                nc.vector.tensor_scalar(s1b[:].bitcast(U32),
                                        s1b[:].bitcast(U32), 0x7FFFFFFF,
                                        scalar2=None, op0=ALU.bitwise_and)
                nc.vector.tensor_scalar(s1b[:].bitcast(U32), s1b[:],
                                        v1s[:, :1], scalar2=None,
                                        op0=ALU.is_lt)
                w1m = mmp.tile([KP, KT * 128], BF16, tag="w1m")
                nc.vector.select(w1m[:], s1b[:].bitcast(U32),
                                 zbf16[:KP].to_broadcast([KP, KT * 128]),
                                 w1b[:])
                hts = []
                for bb in range(NBB):
                    ph = psh.tile([128, BBS], F32, tag="ph")
                    for kt in range(KT):
                        nc.tensor.matmul(
                            ph[:], w1m[:, kt * 128:(kt + 1) * 128],
                            xsb[:, kt * BS + bb * BBS:
                                kt * BS + (bb + 1) * BBS],
                            start=(kt == 0), stop=(kt == KT - 1))
                    ht = hbp.tile([128, BBS], BF16, tag="ht")
                    nc.scalar.activation(ht[:], ph[:], AF.Relu, bias=0.0,
                                         scale=1.0)
                    hts.append(ht)
                w2s = w2m[:, nb * N_OUT:(nb + 1) * N_OUT]
                for bb in range(NBB):
                    nc.tensor.matmul(lgps[bb][:], w2s, hts[bb][:],
                                     start=(nb == 0), stop=(nb == NB - 1),
                                     skip_group_check=True)

            # ================= epilogue: log_softmax =================
            lga = epi.tile([128, 16 * N_OUT], F32, tag="lga")
            for bb in range(NBB):
                lg = epi.tile([N_OUT, BBS], F32, tag="lgc")
                nc.vector.tensor_copy(lg[:], lgps[bb][:])
                for c in range(BBS // 128):
                    g = bb * (BBS // 128) + c
                    pt = psh.tile([128, BBS], F32, tag="ph")
                    nc.tensor.transpose(pt[:, :N_OUT],
                                        lg[:, c * 128:(c + 1) * 128],
                                        ident[:N_OUT, :N_OUT])
                    nc.vector.tensor_copy(lga[:, g * N_OUT:(g + 1) * N_OUT],
                                          pt[:, :N_OUT])
            lga3 = lga[:].rearrange("p (g k) -> p g k", k=N_OUT)
            mx = epi.tile([128, 16], F32, tag="mx")
            nc.vector.tensor_reduce(mx[:], lga3, axis=AX.X, op=ALU.max)
            mxb = mx[:].unsqueeze(2).to_broadcast([128, 16, N_OUT])
            nc.vector.tensor_tensor(lga3, lga3, mxb, op=ALU.subtract)
            ex = epi.tile([128, 16 * N_OUT], F32, tag="ex")
            nc.scalar.activation(ex[:], lga[:], AF.Exp, bias=0.0, scale=1.0)
            se = epi.tile([128, 16], F32, tag="se")
            nc.vector.tensor_reduce(se[:],
                                    ex[:].rearrange("p (g k) -> p g k",
                                                    k=N_OUT),
                                    axis=AX.X, op=ALU.add)
            ls = epi.tile([128, 16], F32, tag="ls")
            nc.scalar.activation(ls[:], se[:], AF.Ln, bias=zb[:, :1],
                                 scale=1.0)
            lsb = ls[:].unsqueeze(2).to_broadcast([128, 16, N_OUT])
            nc.vector.tensor_tensor(lga3, lga3, lsb, op=ALU.subtract)
            for g in range(16):
                nc.sync.dma_start(out[g * 128:(g + 1) * 128, :],
                                  lga[:, g * N_OUT:(g + 1) * N_OUT])
    nc.compile()
    return nc


def _prep_inputs(x, w1, s1, w2, s2):
    bf = ml_dtypes.bfloat16
    w1r = np.ascontiguousarray(
        w1.reshape(NB, 128, KT, KP).transpose(3, 0, 2, 1).reshape(KP, WCOL)
    ).astype(bf)
    s1r = np.ascontiguousarray(
        s1.reshape(NB, 128, KT, KP).transpose(3, 0, 2, 1).reshape(KP, WCOL)
    ).astype(np.float32)
    w2r = np.ascontiguousarray(
        w2.T.reshape(NB, 128, N_OUT).transpose(1, 0, 2).reshape(128,
                                                                NB * N_OUT)
    ).astype(bf)
    s2r = np.ascontiguousarray(
        s2.T.reshape(NB, 128, N_OUT).transpose(1, 0, 2).reshape(128,
                                                                NB * N_OUT)
    ).astype(np.float32)
    in_maps = []
    for cid in range(N_CORES):
        xc = np.ascontiguousarray(
            x[cid * BS:(cid + 1) * BS].T).reshape(KT, KP, BS).astype(bf)
        shc = np.ascontiguousarray(s1r[:, cid * SH:(cid + 1) * SH])
        in_maps.append({"xT": xc, "w1r": w1r, "s1r": s1r, "s1sh": shc,
                        "w2r": w2r, "s2r": s2r})
    return in_maps


def kernel(x, w1, s1, w2, s2):
    x = np.asarray(x); w1 = np.asarray(w1); s1 = np.asarray(s1)
    w2 = np.asarray(w2); s2 = np.asarray(s2)
    if "nc" not in _cache:
        _cache["nc"] = build_program()
    nc = _cache["nc"]
    in_maps = _prep_inputs(x, w1, s1, w2, s2)
    res = run_bass_kernel_spmd(nc, in_maps, list(range(N_CORES)))
    return np.concatenate([res.results[c]["out"] for c in range(N_CORES)],
                          axis=0)


if __name__ == "__main__":
    sys.path.insert(0, "/root/problem")
    from reference import setup_inputs
    inputs = {k: np.asarray(v) for k, v in setup_inputs().items()}
    got = kernel(**inputs)
    print("out", got.shape, got.dtype)
    print(got[:2])
